# revision 26
# baseline (speedup 1.0000x reference)
"""Trainium2 (8 NeuronCores) kernel for a gated-attention transformer block.

Reference computation (per batch b):
    q = x@Wq, [k|v] = x@Wkv, heads=8, dh=64
    attn = softmax(q k^T / 8) v
    out  = (attn * sigmoid(x@Wg + bg)) @ Wo + bo + x
    out  = LayerNorm(out) * gamma + beta

Sharding: 8 cores = 4 batches x 2 sequence-halves. Each core computes
k/v for its full batch (duplicated across the half-pair; avoids any
collective) and q/gates/output for its own 1024 rows. Row order of
keys/values is irrelevant to attention, so each core receives x[b]
rolled so its own rows come first; compile-time indices are then
identical across cores (SPMD-safe).

On-chip layout: activations transposed ([feature, seq]) for projections
and attention; dots computed as dotsT[j, i] with 2x row-tiled matmuls
(K=64 head pairs on PE quadrants), softmax denominator via a ones-column
augmented attn@v matmul (M=65), gating + denominator applied in
transposed layout, final Wo projection back to natural layout for the
residual + LayerNorm tail. All matmuls bf16 with fp32 PSUM accumulation.

Scheduling: projections for head-pair p+1 are emitted interleaved with
attention of pair p so the TensorEngine stays busy while the ScalarEngine
runs the (bottleneck) softmax exponentials. All sigmoids are emitted
before the first exp and the LayerNorm sqrts after the last one, so the
ScalarEngine's activation table is switched exactly twice.
"""

import sys
import os
import numpy as np

for _p in ("/opt/trn_rl_repo", "/root/.axon_site/_ro/trn_rl_repo"):
    if os.path.isdir(_p) and _p not in sys.path:
        sys.path.insert(0, _p)

import concourse.bass as bass
import concourse.tile as tile
from concourse import bacc, mybir
from concourse.bass_utils import run_bass_kernel_spmd
from concourse.masks import make_identity

F32 = mybir.dt.float32
BF16 = mybir.dt.bfloat16
AF = mybir.ActivationFunctionType
OP = mybir.AluOpType

B, N, D, H, DH = 4, 2048, 512, 8, 64
NH = N // 2          # rows owned per core
NJT = N // 128       # 16 key tiles
SCALE = DH ** -0.5   # 0.125
EPS = 1e-5
NCORES = 8


def build_nc(trivial_bo=False, trivial_gb=False):
    nc = bacc.Bacc("TRN2", target_bir_lowering=False, debug=False,
                   num_devices=NCORES)

    xkv = nc.dram_tensor("xkv", [N, D], F32, kind="ExternalInput")
    Wq = nc.dram_tensor("Wq", [D, D], F32, kind="ExternalInput")
    Wk = nc.dram_tensor("Wk", [D, D], F32, kind="ExternalInput")
    Wv = nc.dram_tensor("Wv", [D, D], F32, kind="ExternalInput")
    Wg = nc.dram_tensor("Wg", [D, D], F32, kind="ExternalInput")
    Wo = nc.dram_tensor("Wo", [D, D], F32, kind="ExternalInput")
    bg = nc.dram_tensor("bg", [D], F32, kind="ExternalInput")
    bo = nc.dram_tensor("bo", [D], F32, kind="ExternalInput")
    gamma = nc.dram_tensor("gamma", [D], F32, kind="ExternalInput")
    beta = nc.dram_tensor("beta", [D], F32, kind="ExternalInput")
    out = nc.dram_tensor("out", [NH, D], F32, kind="ExternalOutput")

    def bcast_ap(t, n):
        return bass.AP(tensor=t, offset=0, ap=[[0, 128], [1, n]])

    with tile.TileContext(nc) as tc:
        with tc.tile_pool(name="consts", bufs=1) as consts, \
             tc.tile_pool(name="wpool", bufs=1) as wpool, \
             tc.tile_pool(name="acts", bufs=1) as acts, \
             tc.tile_pool(name="stage", bufs=2) as stage, \
             tc.tile_pool(name="prpool", bufs=4) as prpool, \
             tc.tile_pool(name="ppool", bufs=2, space="PSUM") as ppool, \
             tc.tile_pool(name="papool", bufs=2, space="PSUM") as papool, \
             tc.tile_pool(name="pmisc", bufs=2, space="PSUM") as pmisc:

            # ---- constants ----
            ident = consts.tile([128, 128], BF16)
            make_identity(nc, ident[:])
            bg_t = consts.tile([64, H], F32)
            nc.sync.dma_start(bg_t[:], bg.ap().rearrange("(h p) -> p h", p=64))
            bo_b = consts.tile([128, D], F32)
            nc.sync.dma_start(bo_b[:], bcast_ap(bo, D))
            gam_b = consts.tile([128, D], F32)
            nc.sync.dma_start(gam_b[:], bcast_ap(gamma, D))
            bet_b = consts.tile([128, D], F32)
            nc.sync.dma_start(bet_b[:], bcast_ap(beta, D))
            eps_t = consts.tile([128, 1], F32)
            nc.vector.memset(eps_t[:], EPS)

            # ---- weights: load fp32 in 128-row chunks, cast to bf16.
            #      Weight DMAs ride the sync queue; x DMAs ride the scalar
            #      queue so the two streams overlap. ----
            w_bf = {}
            for name, t in (("Wq", Wq), ("Wk", Wk), ("Wv", Wv), ("Wg", Wg)):
                wb = wpool.tile([128, 4, D], BF16, tag=f"w_{name}")
                for kc in range(4):
                    ws = stage.tile([128, D], F32, tag="wstage", bufs=4)
                    nc.sync.dma_start(ws[:], t[kc * 128:(kc + 1) * 128, :])
                    nc.vector.tensor_copy(wb[:, kc, :], ws[:])
                w_bf[name] = wb
            wo_b = wpool.tile([64, H, D], BF16)
            for h in range(H):
                ws = stage.tile([128, D], F32, tag="wstage", bufs=4)
                nc.sync.dma_start(ws[0:64, :], Wo[h * 64:(h + 1) * 64, :])
                nc.vector.tensor_copy(wo_b[:, h, :], ws[0:64, :])

            # ---- tensors for x / projections ----
            xT = acts.tile([128, 4, N], BF16)
            sigT = acts.tile([64, H, NH], BF16)
            qT = acts.tile([128, 4, NH], BF16)
            kT = acts.tile([128, 4, N], BF16)
            v3 = acts.tile([128, NJT, H, DH + 1], BF16)
            nc.vector.memset(v3[:, :, :, DH:DH + 1], 1.0)

            def gates_unit(m, ic):
                def emit():
                    pm = pmisc.tile([128, 512], F32, tag="m")
                    for kc in range(4):
                        nc.tensor.matmul(pm[:], w_bf["Wg"][:, kc, m * 128:(m + 1) * 128],
                                         xT[:, kc, ic * 512:(ic + 1) * 512],
                                         start=(kc == 0), stop=(kc == 3))
                    nc.scalar.activation(sigT[:, 2 * m, ic * 512:(ic + 1) * 512],
                                         pm[0:64, :], AF.Sigmoid,
                                         bias=bg_t[:, 2 * m:2 * m + 1])
                    nc.scalar.activation(sigT[:, 2 * m + 1, ic * 512:(ic + 1) * 512],
                                         pm[64:128, :], AF.Sigmoid,
                                         bias=bg_t[:, 2 * m + 1:2 * m + 2])
                return emit

            def qt_unit(m, ic):
                def emit():
                    pm = pmisc.tile([128, 512], F32, tag="m")
                    for kc in range(4):
                        nc.tensor.matmul(pm[:], w_bf["Wq"][:, kc, m * 128:(m + 1) * 128],
                                         xT[:, kc, ic * 512:(ic + 1) * 512],
                                         start=(kc == 0), stop=(kc == 3))
                    nc.vector.tensor_copy(qT[:, m, ic * 512:(ic + 1) * 512], pm[:])
                return emit

            def kt_unit(m, ic):
                def emit():
                    pm = pmisc.tile([128, 512], F32, tag="m")
                    for kc in range(4):
                        nc.tensor.matmul(pm[:], w_bf["Wk"][:, kc, m * 128:(m + 1) * 128],
                                         xT[:, kc, ic * 512:(ic + 1) * 512],
                                         start=(kc == 0), stop=(kc == 3))
                    nc.vector.tensor_copy(kT[:, m, ic * 512:(ic + 1) * 512], pm[:])
                return emit

            def v_unit(jt):
                def emit():
                    pm = pmisc.tile([128, 512], F32, tag="m")
                    for kc in range(4):
                        nc.tensor.matmul(pm[:], xT[:, kc, jt * 128:(jt + 1) * 128],
                                         w_bf["Wv"][:, kc, :],
                                         start=(kc == 0), stop=(kc == 3))
                    nc.vector.tensor_copy(
                        v3[:, jt, :, 0:DH],
                        pm[:].rearrange("p (h d) -> p h d", h=H))
                return emit

            # ---- x: load, cast, transpose; prelude projection units are
            #      emitted as soon as the xT columns they read exist, so
            #      gates/q/k/v overlap the transpose pipeline and attention
            #      can start while the tail of x is still being transposed.
            #      All sigmoids stay before the first exp (one table switch).
            prelude = {
                3: [kt_unit(0, 0), gates_unit(0, 0), gates_unit(1, 0)],
                4: [gates_unit(2, 0), gates_unit(3, 0), v_unit(0)],
                5: [qt_unit(0, 0), v_unit(1)],
                7: [kt_unit(0, 1), gates_unit(0, 1), gates_unit(1, 1)],
                8: [gates_unit(2, 1), gates_unit(3, 1), qt_unit(0, 1)],
                11: [kt_unit(0, 2)],
                15: [kt_unit(0, 3)],
            }
            for nt in range(N // 128):
                xs = stage.tile([128, D], F32, tag="xstage", bufs=3)
                nc.scalar.dma_start(xs[:], xkv[nt * 128:(nt + 1) * 128, :])
                xb = stage.tile([128, D], BF16, tag="xbf")
                nc.vector.tensor_copy(xb[:], xs[:])
                for kc in range(4):
                    pt = pmisc.tile([128, 128], BF16, tag="m")
                    nc.tensor.transpose(pt[:], xb[:, kc * 128:(kc + 1) * 128], ident[:])
                    nc.vector.tensor_copy(xT[:, kc, nt * 128:(nt + 1) * 128], pt[:])
                for unit in prelude.get(nt, []):
                    unit()

            # during pair p's attention, emit projections for pair p+1
            # (v3 for the remaining jt is finished inside pair-0 ic=0,
            # pipelined two key-tiles ahead of its consumer)
            queues = {
                0: [qt_unit(1, ic) for ic in range(2)]
                   + [kt_unit(1, ic) for ic in range(4)],
                1: [qt_unit(2, ic) for ic in range(2)]
                   + [kt_unit(2, ic) for ic in range(4)],
                2: [qt_unit(3, ic) for ic in range(2)]
                   + [kt_unit(3, ic) for ic in range(4)],
                3: None,  # filled per-ic below: Wo/LN for it 0..3 during ic=1
            }

            # ---- attention, per head pair ----
            gatedT = acts.tile([64, H, NH], BF16)

            def wo_unit(it, use_att=False):
                def emit():
                    xres = stage.tile([128, D], F32, tag=f"xres{it % 4}")
                    nc.scalar.dma_start(xres[:], xkv[it * 128:(it + 1) * 128, :])
                    pool_w, ptag = (papool, "att") if use_att else (pmisc, "m")
                    pw = pool_w.tile([128, 512], F32, tag=ptag)
                    for h in range(H):
                        nc.tensor.matmul(pw[:], gatedT[:, h, it * 128:(it + 1) * 128],
                                         wo_b[:, h, :], start=(h == 0),
                                         stop=(h == H - 1))
                    y = stage.tile([128, D], F32, tag="y")
                    nc.vector.tensor_add(y[:], pw[:], xres[:])
                    if not trivial_bo:
                        nc.vector.tensor_add(y[:], y[:], bo_b[:])
                    st = stage.tile([128, 6], F32, tag="st")
                    nc.vector.bn_stats(st[:], y[:])
                    mv = stage.tile([128, 2], F32, tag="mv")
                    nc.vector.bn_aggr(mv[:], st[:])
                    ve = stage.tile([128, 1], F32, tag="ve")
                    nc.vector.tensor_add(ve[:], mv[:, 1:2], eps_t[:])
                    nc.vector.reciprocal(ve[:], ve[:])
                    nc.scalar.activation(ve[:], ve[:], AF.Sqrt)
                    z = stage.tile([128, D], F32, tag="z")
                    nc.vector.tensor_scalar(z[:], y[:], mv[:, 0:1], ve[:],
                                            OP.subtract, OP.mult)
                    if not trivial_gb:
                        nc.vector.tensor_mul(z[:], z[:], gam_b[:])
                        nc.vector.tensor_add(z[:], z[:], bet_b[:])
                    nc.sync.dma_start(out[it * 128:(it + 1) * 128, :], z[:])
                return emit

            for p in range(4):
                work = queues[p] or []
                wi = 0
                for ic in range(NH // 512):
                    if p == 3 and ic == 1:
                        work = [wo_unit(it) for it in range(4)]
                        wi = 0
                    pe_ = papool.tile([128, 512], F32, tag="att")
                    po_ = papool.tile([128, 512], F32, tag="att")
                    for jt in range(NJT):
                        if p == 0 and ic == 0 and jt + 2 < NJT:
                            v_unit(jt + 2)()
                        elif wi < len(work) and (jt % 2 == 0 or wi > len(work) - 3):
                            work[wi]()
                            wi += 1
                        pd = ppool.tile([128, 1024], F32)
                        nc.tensor.matmul(pd[:, 0:512],
                                         kT[0:64, p, jt * 128:(jt + 1) * 128],
                                         qT[0:64, p, ic * 512:(ic + 1) * 512],
                                         start=True, stop=True,
                                         tile_position=(0, 0))
                        nc.tensor.matmul(pd[:, 512:1024],
                                         kT[64:128, p, jt * 128:(jt + 1) * 128],
                                         qT[64:128, p, ic * 512:(ic + 1) * 512],
                                         start=True, stop=True,
                                         tile_position=(64, 0))
                        pr = prpool.tile([128, 2, 512], BF16, tag="pr")
                        nc.scalar.activation(
                            pr[:], pd[:].rearrange("p (h x) -> p h x", h=2),
                            AF.Exp, scale=SCALE)
                        nc.tensor.matmul(pe_[0:65, :], v3[:, jt, 2 * p, :],
                                         pr[:, 0, :],
                                         start=(jt == 0), stop=(jt == NJT - 1))
                        nc.tensor.matmul(po_[0:65, :], v3[:, jt, 2 * p + 1, :],
                                         pr[:, 1, :],
                                         start=(jt == 0), stop=(jt == NJT - 1))
                    for hh, ph in ((2 * p, pe_), (2 * p + 1, po_)):
                        # evacuate PSUM fast (frees the accumulator bank for
                        # the next ic), then gate from SBUF off-critical-path
                        raw = stage.tile([65, 512], F32, tag="praw", bufs=4)
                        nc.vector.tensor_copy(raw[:], ph[0:65, :])
                        r0 = stage.tile([1, 512], F32, tag="r0")
                        nc.vector.reciprocal(r0[:], raw[64:65, :])
                        rb = stage.tile([64, 512], F32, tag="rb")
                        nc.gpsimd.partition_broadcast(rb[:], r0[:])
                        tmp = stage.tile([64, 512], F32, tag="tmp")
                        nc.vector.tensor_mul(tmp[:], raw[0:64, :], rb[:])
                        nc.vector.tensor_mul(gatedT[:, hh, ic * 512:(ic + 1) * 512],
                                             tmp[:], sigT[:, hh, ic * 512:(ic + 1) * 512])

            # ---- remaining Wo + LayerNorm tail units (it 4..7; 0..3 were
            #      interleaved into pair-3 attention) ----
            for it in range(4, NH // 128):
                wo_unit(it, use_att=(it % 2 == 1))()

    nc.compile()
    return nc


_NC_CACHE = {}


def _get_nc(trivial_bo=False, trivial_gb=False):
    key = (trivial_bo, trivial_gb)
    if key not in _NC_CACHE:
        _NC_CACHE[key] = build_nc(*key)
    return _NC_CACHE[key]


def kernel(**inputs) -> np.ndarray:
    x = np.asarray(inputs["x"], dtype=np.float32)
    Wq = np.ascontiguousarray(np.asarray(inputs["Wq"], dtype=np.float32))
    Wkv = np.asarray(inputs["Wkv"], dtype=np.float32)
    Wk = np.ascontiguousarray(Wkv[:, :D])
    Wv = np.ascontiguousarray(Wkv[:, D:])
    Wg = np.ascontiguousarray(np.asarray(inputs["Wg"], dtype=np.float32))
    Wo = np.ascontiguousarray(np.asarray(inputs["Wo"], dtype=np.float32))
    bg = np.ascontiguousarray(np.asarray(inputs["bg"], dtype=np.float32))
    bo = np.ascontiguousarray(np.asarray(inputs["bo"], dtype=np.float32))
    gamma = np.ascontiguousarray(np.asarray(inputs["gamma"], dtype=np.float32))
    beta = np.ascontiguousarray(np.asarray(inputs["beta"], dtype=np.float32))

    trivial_bo = bool(np.all(bo == 0.0))
    trivial_gb = bool(np.all(gamma == 1.0) and np.all(beta == 0.0))
    nc = _get_nc(trivial_bo, trivial_gb)
    in_maps = []
    for c in range(NCORES):
        b, half = c // 2, c % 2
        rolled = np.ascontiguousarray(np.roll(x[b], -half * NH, axis=0))
        in_maps.append({"xkv": rolled, "Wq": Wq, "Wk": Wk, "Wv": Wv,
                        "Wg": Wg, "Wo": Wo, "bg": bg, "bo": bo,
                        "gamma": gamma, "beta": beta})
    res = run_bass_kernel_spmd(nc, in_maps, core_ids=list(range(NCORES)))
    out = np.empty((B, N, D), dtype=np.float32)
    for c in range(NCORES):
        b, half = c // 2, c % 2
        out[b, half * NH:(half + 1) * NH] = res.results[c]["out"]
    return out


# revision 38
# speedup vs baseline: 1.0437x; 1.0437x over previous
"""Trainium2 (8 NeuronCores) kernel for a gated-attention transformer block.

Reference computation (per batch b):
    q = x@Wq, [k|v] = x@Wkv, heads=8, dh=64
    attn = softmax(q k^T / 8) v
    out  = (attn * sigmoid(x@Wg + bg)) @ Wo + bo + x
    out  = LayerNorm(out) * gamma + beta

Sharding: 8 cores = 4 batches x 2 sequence-halves. Each core computes
k/v for its full batch (duplicated across the half-pair; avoids any
collective) and q/gates/output for its own 1024 rows. Row order of
keys/values is irrelevant to attention, so each core receives x[b]
rolled so its own rows come first; compile-time indices are then
identical across cores (SPMD-safe).

On-chip layout: activations transposed ([feature, seq]) for projections
and attention; dots computed as dotsT[j, i] with 2x row-tiled matmuls
(K=64 head pairs on PE quadrants), softmax denominator via a ones-column
augmented attn@v matmul (M=65), gating + denominator applied in
transposed layout, final Wo projection back to natural layout for the
residual + LayerNorm tail. All matmuls bf16 with fp32 PSUM accumulation.

Scheduling: projections for head-pair p+1 are emitted interleaved with
attention of pair p so the TensorEngine stays busy while the ScalarEngine
runs the (bottleneck) softmax exponentials. All sigmoids are emitted
before the first exp and the LayerNorm sqrts after the last one, so the
ScalarEngine's activation table is switched exactly twice.
"""

import sys
import os
import numpy as np

for _p in ("/opt/trn_rl_repo", "/root/.axon_site/_ro/trn_rl_repo"):
    if os.path.isdir(_p) and _p not in sys.path:
        sys.path.insert(0, _p)

import concourse.bass as bass
import concourse.tile as tile
from concourse import bacc, mybir
from concourse.bass_utils import run_bass_kernel_spmd
from concourse.masks import make_identity

F32 = mybir.dt.float32
BF16 = mybir.dt.bfloat16
AF = mybir.ActivationFunctionType
OP = mybir.AluOpType

B, N, D, H, DH = 4, 2048, 512, 8, 64
NH = N // 2          # rows owned per core
NJT = N // 128       # 16 key tiles
SCALE = DH ** -0.5   # 0.125
EPS = 1e-5
NCORES = 8


def build_nc(trivial_bo=False, trivial_gb=False):
    nc = bacc.Bacc("TRN2", target_bir_lowering=False, debug=False,
                   num_devices=NCORES)

    xkv = nc.dram_tensor("xkv", [N, D], F32, kind="ExternalInput")
    Wq = nc.dram_tensor("Wq", [D, D], F32, kind="ExternalInput")
    Wk = nc.dram_tensor("Wk", [D, D], F32, kind="ExternalInput")
    Wv = nc.dram_tensor("Wv", [D, D], F32, kind="ExternalInput")
    Wg = nc.dram_tensor("Wg", [D, D], F32, kind="ExternalInput")
    Wo = nc.dram_tensor("Wo", [D, D], F32, kind="ExternalInput")
    bg = nc.dram_tensor("bg", [D], F32, kind="ExternalInput")
    bo = nc.dram_tensor("bo", [D], F32, kind="ExternalInput")
    gamma = nc.dram_tensor("gamma", [D], F32, kind="ExternalInput")
    beta = nc.dram_tensor("beta", [D], F32, kind="ExternalInput")
    out = nc.dram_tensor("out", [NH, D], F32, kind="ExternalOutput")

    def bcast_ap(t, n):
        return bass.AP(tensor=t, offset=0, ap=[[0, 128], [1, n]])

    with tile.TileContext(nc) as tc:
        with tc.tile_pool(name="consts", bufs=1) as consts, \
             tc.tile_pool(name="wpool", bufs=1) as wpool, \
             tc.tile_pool(name="acts", bufs=1) as acts, \
             tc.tile_pool(name="stage", bufs=2) as stage, \
             tc.tile_pool(name="prpool", bufs=4) as prpool, \
             tc.tile_pool(name="ppool", bufs=2, space="PSUM") as ppool, \
             tc.tile_pool(name="papool", bufs=2, space="PSUM") as papool, \
             tc.tile_pool(name="pmisc", bufs=2, space="PSUM") as pmisc:

            # ---- constants ----
            ident = consts.tile([128, 128], BF16)
            make_identity(nc, ident[:])
            bg_t = consts.tile([64, H], F32)
            nc.sync.dma_start(bg_t[:], bg.ap().rearrange("(h p) -> p h", p=64))
            bo_b = consts.tile([128, D], F32)
            nc.sync.dma_start(bo_b[:], bcast_ap(bo, D))
            gam_b = consts.tile([128, D], F32)
            nc.sync.dma_start(gam_b[:], bcast_ap(gamma, D))
            bet_b = consts.tile([128, D], F32)
            nc.sync.dma_start(bet_b[:], bcast_ap(beta, D))
            eps_t = consts.tile([128, 1], F32)
            nc.vector.memset(eps_t[:], EPS)

            # ---- weights: load fp32 in 128-row chunks, cast to bf16.
            #      Weight DMAs ride the sync queue; x DMAs ride the scalar
            #      queue so the two streams overlap. ----
            w_bf = {}
            for name, t in (("Wk", Wk), ("Wq", Wq), ("Wv", Wv), ("Wg", Wg)):
                wb = wpool.tile([128, 4, D], BF16, tag=f"w_{name}")
                for kc in range(4):
                    ws = stage.tile([128, D], F32, tag="wstage", bufs=4)
                    nc.sync.dma_start(ws[:], t[kc * 128:(kc + 1) * 128, :])
                    nc.vector.tensor_copy(wb[:, kc, :], ws[:])
                w_bf[name] = wb
            wo_b = wpool.tile([64, H, D], BF16)
            for h in range(H):
                ws = stage.tile([128, D], F32, tag="wostage", bufs=2)
                nc.gpsimd.dma_start(ws[0:64, :], Wo[h * 64:(h + 1) * 64, :])
                nc.vector.tensor_copy(wo_b[:, h, :], ws[0:64, :])
            nbg = consts.tile([128, 4], F32)
            nc.sync.dma_start(nbg[:], bg.ap().rearrange("(m p) -> p m", p=128))
            nc.vector.tensor_scalar_mul(nbg[:], nbg[:], -1.0)

            # ---- tensors for x / projections ----
            xT = acts.tile([128, 4, N], BF16)
            sigT = acts.tile([64, H, NH], BF16)
            qT = acts.tile([128, 4, NH], BF16)
            kT = acts.tile([128, 4, N], BF16)
            v3 = acts.tile([128, NJT, H, DH + 1], BF16)
            nc.vector.memset(v3[:, :, :, DH:DH + 1], 1.0)

            def gates_unit(m, ic):
                # sigmoid(g+bg) = 1/(1+exp(-(g+bg))) -- uses the Exp table so
                # these can interleave freely with the attention exps
                def emit():
                    pm = pmisc.tile([128, 512], F32, tag="m")
                    for kc in range(4):
                        nc.tensor.matmul(pm[:], w_bf["Wg"][:, kc, m * 128:(m + 1) * 128],
                                         xT[:, kc, ic * 512:(ic + 1) * 512],
                                         start=(kc == 0), stop=(kc == 3))
                    e = stage.tile([128, 512], F32, tag="gexp")
                    nc.scalar.activation(e[:], pm[:], AF.Exp, scale=-1.0,
                                         bias=nbg[:, m:m + 1])
                    nc.vector.tensor_scalar_add(e[:], e[:], 1.0)
                    sp = stage.tile([128, 512], F32, tag="gsig")
                    nc.vector.reciprocal(sp[:], e[:])
                    nc.vector.tensor_copy(sigT[:, 2 * m, ic * 512:(ic + 1) * 512],
                                          sp[0:64, :])
                    nc.vector.tensor_copy(sigT[:, 2 * m + 1, ic * 512:(ic + 1) * 512],
                                          sp[64:128, :])
                return emit

            def qt_unit(m, ic):
                def emit():
                    pm = pmisc.tile([128, 512], F32, tag="m")
                    for kc in range(4):
                        nc.tensor.matmul(pm[:], w_bf["Wq"][:, kc, m * 128:(m + 1) * 128],
                                         xT[:, kc, ic * 512:(ic + 1) * 512],
                                         start=(kc == 0), stop=(kc == 3))
                    nc.vector.tensor_copy(qT[:, m, ic * 512:(ic + 1) * 512], pm[:])
                return emit

            def kt_unit(m, ic):
                def emit():
                    pm = pmisc.tile([128, 512], F32, tag="m")
                    for kc in range(4):
                        nc.tensor.matmul(pm[:], w_bf["Wk"][:, kc, m * 128:(m + 1) * 128],
                                         xT[:, kc, ic * 512:(ic + 1) * 512],
                                         start=(kc == 0), stop=(kc == 3))
                    nc.vector.tensor_copy(kT[:, m, ic * 512:(ic + 1) * 512], pm[:])
                return emit

            def v_unit(jt):
                def emit():
                    pm = pmisc.tile([128, 512], F32, tag="m")
                    for kc in range(4):
                        nc.tensor.matmul(pm[:], xT[:, kc, jt * 128:(jt + 1) * 128],
                                         w_bf["Wv"][:, kc, :],
                                         start=(kc == 0), stop=(kc == 3))
                    nc.vector.tensor_copy(
                        v3[:, jt, :, 0:DH],
                        pm[:].rearrange("p (h d) -> p h d", h=H))
                return emit

            # ---- x: load, cast, transpose; prelude projection units are
            #      emitted as soon as the xT columns they read exist, so
            #      gates/q/k/v overlap the transpose pipeline and attention
            #      can start while the tail of x is still being transposed.
            #      All sigmoids stay before the first exp (one table switch).
            prelude = {
                3: [kt_unit(0, 0)],
                4: [v_unit(0)],
                5: [qt_unit(0, 0), v_unit(1)],
                7: [kt_unit(0, 1)],
                8: [qt_unit(0, 1)],
                11: [kt_unit(0, 2)],
                15: [kt_unit(0, 3)],
            }
            for nt in range(N // 128):
                xs = stage.tile([128, D], F32, tag="xstage", bufs=4)
                dma_eng = nc.scalar if nt % 2 == 0 else nc.gpsimd
                dma_eng.dma_start(xs[:], xkv[nt * 128:(nt + 1) * 128, :])
                xb = stage.tile([128, D], BF16, tag="xbf")
                nc.vector.tensor_copy(xb[:], xs[:])
                for kc in range(4):
                    pt = pmisc.tile([128, 128], BF16, tag="m")
                    nc.tensor.transpose(pt[:], xb[:, kc * 128:(kc + 1) * 128], ident[:])
                    nc.vector.tensor_copy(xT[:, kc, nt * 128:(nt + 1) * 128], pt[:])
                for unit in prelude.get(nt, []):
                    unit()

            # during pair p's attention, emit projections for pair p+1
            # (v3 for the remaining jt is finished inside pair-0 ic=0,
            # pipelined two key-tiles ahead of its consumer)
            queues = {
                0: [gates_unit(1, 0), gates_unit(1, 1)]
                   + [qt_unit(1, ic) for ic in range(2)]
                   + [kt_unit(1, ic) for ic in range(4)],
                1: [gates_unit(2, 0), gates_unit(2, 1)]
                   + [qt_unit(2, ic) for ic in range(2)]
                   + [kt_unit(2, ic) for ic in range(4)],
                2: [gates_unit(3, 0), gates_unit(3, 1)]
                   + [qt_unit(3, ic) for ic in range(2)]
                   + [kt_unit(3, ic) for ic in range(4)],
                3: None,  # filled per-ic below: Wo/LN for it 0..3 during ic=1
            }

            # ---- attention, per head pair ----
            gatedT = acts.tile([64, H, NH], BF16)

            def wo_unit(it, psum="m"):
                def emit():
                    xres = stage.tile([128, D], F32, tag=f"xres{it % 4}")
                    nc.scalar.dma_start(xres[:], xkv[it * 128:(it + 1) * 128, :])
                    if psum == "att":
                        pw = papool.tile([128, 512], F32, tag="att")
                    elif psum == "pd":
                        pw_full = ppool.tile([128, 1024], F32, tag="pd")
                        pw = pw_full[:, 0:512]
                    else:
                        pw = pmisc.tile([128, 512], F32, tag="m")
                    for h in range(H):
                        nc.tensor.matmul(pw[:], gatedT[:, h, it * 128:(it + 1) * 128],
                                         wo_b[:, h, :], start=(h == 0),
                                         stop=(h == H - 1))
                    y = stage.tile([128, D], F32, tag="y")
                    nc.vector.tensor_add(y[:], pw[:], xres[:])
                    if not trivial_bo:
                        nc.vector.tensor_add(y[:], y[:], bo_b[:])
                    st = stage.tile([128, 6], F32, tag="st")
                    nc.vector.bn_stats(st[:], y[:])
                    mv = stage.tile([128, 2], F32, tag="mv")
                    nc.vector.bn_aggr(mv[:], st[:])
                    ve = stage.tile([128, 1], F32, tag="ve")
                    nc.vector.tensor_add(ve[:], mv[:, 1:2], eps_t[:])
                    nc.vector.reciprocal(ve[:], ve[:])
                    nc.scalar.activation(ve[:], ve[:], AF.Sqrt)
                    z = stage.tile([128, D], F32, tag="z")
                    nc.vector.tensor_scalar(z[:], y[:], mv[:, 0:1], ve[:],
                                            OP.subtract, OP.mult)
                    if not trivial_gb:
                        nc.vector.tensor_mul(z[:], z[:], gam_b[:])
                        nc.vector.tensor_add(z[:], z[:], bet_b[:])
                    nc.sync.dma_start(out[it * 128:(it + 1) * 128, :], z[:])
                return emit

            for p in range(4):
                work = queues[p] or []
                wi = 0
                for ic in range(NH // 512):
                    if p == 3 and ic == 1:
                        work = [wo_unit(it) for it in range(4)]
                        wi = 0
                    pe_ = papool.tile([128, 512], F32, tag="att")
                    po_ = papool.tile([128, 512], F32, tag="att")

                    def dots_step(jt):
                        pd = ppool.tile([128, 1024], F32)
                        nc.tensor.matmul(pd[:, 0:512],
                                         kT[0:64, p, jt * 128:(jt + 1) * 128],
                                         qT[0:64, p, ic * 512:(ic + 1) * 512],
                                         start=True, stop=True,
                                         tile_position=(0, 0))
                        nc.tensor.matmul(pd[:, 512:1024],
                                         kT[64:128, p, jt * 128:(jt + 1) * 128],
                                         qT[64:128, p, ic * 512:(ic + 1) * 512],
                                         start=True, stop=True,
                                         tile_position=(64, 0))
                        return pd

                    # software pipeline: dots for jt+1 issue on the PE before
                    # the attnVs of jt, which wait on the exp of jt
                    pd_cur = dots_step(0)
                    for jt in range(NJT):
                        pr = prpool.tile([128, 2, 512], BF16, tag="pr")
                        nc.scalar.activation(
                            pr[:], pd_cur[:].rearrange("p (h x) -> p h x", h=2),
                            AF.Exp, scale=SCALE)
                        if jt + 1 < NJT:
                            pd_cur = dots_step(jt + 1)
                        if p == 0 and ic == 0:
                            if jt + 2 < NJT:
                                v_unit(jt + 2)()
                            elif jt == NJT - 2:
                                gates_unit(0, 0)()
                            else:
                                gates_unit(0, 1)()
                        elif wi < len(work) and (jt % 2 == 0 or wi > len(work) - 3):
                            work[wi]()
                            wi += 1
                        nc.tensor.matmul(pe_[0:65, :], v3[:, jt, 2 * p, :],
                                         pr[:, 0, :],
                                         start=(jt == 0), stop=(jt == NJT - 1))
                        nc.tensor.matmul(po_[0:65, :], v3[:, jt, 2 * p + 1, :],
                                         pr[:, 1, :],
                                         start=(jt == 0), stop=(jt == NJT - 1))
                    for hh, ph in ((2 * p, pe_), (2 * p + 1, po_)):
                        # evacuate PSUM fast (frees the accumulator bank for
                        # the next ic), then gate from SBUF off-critical-path
                        raw = stage.tile([65, 512], F32, tag="praw", bufs=4)
                        nc.vector.tensor_copy(raw[:], ph[0:65, :])
                        r0 = stage.tile([1, 512], F32, tag="r0")
                        nc.vector.reciprocal(r0[:], raw[64:65, :])
                        rb = stage.tile([64, 512], F32, tag="rb")
                        nc.gpsimd.partition_broadcast(rb[:], r0[:])
                        tmp = stage.tile([64, 512], F32, tag="tmp")
                        nc.vector.tensor_mul(tmp[:], raw[0:64, :], rb[:])
                        nc.vector.tensor_mul(gatedT[:, hh, ic * 512:(ic + 1) * 512],
                                             tmp[:], sigT[:, hh, ic * 512:(ic + 1) * 512])

            # ---- remaining Wo + LayerNorm tail units (it 4..7; 0..3 were
            #      interleaved into pair-3 attention). Three PSUM slots
            #      (pmisc/papool/ppool) keep the it-tiles pipelined. ----
            for it, ps in ((4, "m"), (5, "att"), (6, "pd"), (7, "m")):
                wo_unit(it, psum=ps)()

    nc.compile()
    return nc


_NC_CACHE = {}


def _get_nc(trivial_bo=False, trivial_gb=False):
    key = (trivial_bo, trivial_gb)
    if key not in _NC_CACHE:
        _NC_CACHE[key] = build_nc(*key)
    return _NC_CACHE[key]


def kernel(**inputs) -> np.ndarray:
    x = np.asarray(inputs["x"], dtype=np.float32)
    Wq = np.ascontiguousarray(np.asarray(inputs["Wq"], dtype=np.float32))
    Wkv = np.asarray(inputs["Wkv"], dtype=np.float32)
    Wk = np.ascontiguousarray(Wkv[:, :D])
    Wv = np.ascontiguousarray(Wkv[:, D:])
    Wg = np.ascontiguousarray(np.asarray(inputs["Wg"], dtype=np.float32))
    Wo = np.ascontiguousarray(np.asarray(inputs["Wo"], dtype=np.float32))
    bg = np.ascontiguousarray(np.asarray(inputs["bg"], dtype=np.float32))
    bo = np.ascontiguousarray(np.asarray(inputs["bo"], dtype=np.float32))
    gamma = np.ascontiguousarray(np.asarray(inputs["gamma"], dtype=np.float32))
    beta = np.ascontiguousarray(np.asarray(inputs["beta"], dtype=np.float32))

    trivial_bo = bool(np.all(bo == 0.0))
    trivial_gb = bool(np.all(gamma == 1.0) and np.all(beta == 0.0))
    nc = _get_nc(trivial_bo, trivial_gb)
    in_maps = []
    for c in range(NCORES):
        b, half = c // 2, c % 2
        rolled = np.ascontiguousarray(np.roll(x[b], -half * NH, axis=0))
        in_maps.append({"xkv": rolled, "Wq": Wq, "Wk": Wk, "Wv": Wv,
                        "Wg": Wg, "Wo": Wo, "bg": bg, "bo": bo,
                        "gamma": gamma, "beta": beta})
    res = run_bass_kernel_spmd(nc, in_maps, core_ids=list(range(NCORES)))
    out = np.empty((B, N, D), dtype=np.float32)
    for c in range(NCORES):
        b, half = c // 2, c % 2
        out[b, half * NH:(half + 1) * NH] = res.results[c]["out"]
    return out


# revision 41
# speedup vs baseline: 1.0753x; 1.0302x over previous
"""Trainium2 (8 NeuronCores) kernel for a gated-attention transformer block.

Reference computation (per batch b):
    q = x@Wq, [k|v] = x@Wkv, heads=8, dh=64
    attn = softmax(q k^T / 8) v
    out  = (attn * sigmoid(x@Wg + bg)) @ Wo + bo + x
    out  = LayerNorm(out) * gamma + beta

Sharding: 8 cores = 4 batches x 2 sequence-halves. Each core computes
k/v for its full batch (duplicated across the half-pair; avoids any
collective) and q/gates/output for its own 1024 rows. Row order of
keys/values is irrelevant to attention, so each core receives x[b]
rolled so its own rows come first; compile-time indices are then
identical across cores (SPMD-safe).

On-chip layout: activations transposed ([feature, seq]) for projections
and attention; dots computed as dotsT[j, i] with 2x row-tiled matmuls
(K=64 head pairs on PE quadrants), softmax denominator via a ones-column
augmented attn@v matmul (M=65), gating + denominator applied in
transposed layout, final Wo projection back to natural layout for the
residual + LayerNorm tail. All matmuls bf16 with fp32 PSUM accumulation.

Scheduling: projections for head-pair p+1 are emitted interleaved with
attention of pair p so the TensorEngine stays busy while the ScalarEngine
runs the (bottleneck) softmax exponentials. All sigmoids are emitted
before the first exp and the LayerNorm sqrts after the last one, so the
ScalarEngine's activation table is switched exactly twice.
"""

import sys
import os
import numpy as np

for _p in ("/opt/trn_rl_repo", "/root/.axon_site/_ro/trn_rl_repo"):
    if os.path.isdir(_p) and _p not in sys.path:
        sys.path.insert(0, _p)

import concourse.bass as bass
import concourse.tile as tile
from concourse import bacc, mybir
from concourse.bass_utils import run_bass_kernel_spmd
from concourse.masks import make_identity

F32 = mybir.dt.float32
BF16 = mybir.dt.bfloat16
AF = mybir.ActivationFunctionType
OP = mybir.AluOpType

B, N, D, H, DH = 4, 2048, 512, 8, 64
NH = N // 2          # rows owned per core
NJT = N // 128       # 16 key tiles
SCALE = DH ** -0.5   # 0.125
EPS = 1e-5
NCORES = 8


def build_nc(trivial_bo=False, trivial_gb=False):
    nc = bacc.Bacc("TRN2", target_bir_lowering=False, debug=False,
                   num_devices=NCORES)

    xkv = nc.dram_tensor("xkv", [N, D], F32, kind="ExternalInput")
    Wq = nc.dram_tensor("Wq", [D, D], F32, kind="ExternalInput")
    Wk = nc.dram_tensor("Wk", [D, D], F32, kind="ExternalInput")
    Wv = nc.dram_tensor("Wv", [D, D], F32, kind="ExternalInput")
    Wg = nc.dram_tensor("Wg", [D, D], F32, kind="ExternalInput")
    Wo = nc.dram_tensor("Wo", [D, D], F32, kind="ExternalInput")
    bg = nc.dram_tensor("bg", [D], F32, kind="ExternalInput")
    bo = nc.dram_tensor("bo", [D], F32, kind="ExternalInput")
    gamma = nc.dram_tensor("gamma", [D], F32, kind="ExternalInput")
    beta = nc.dram_tensor("beta", [D], F32, kind="ExternalInput")
    out = nc.dram_tensor("out", [NH, D], F32, kind="ExternalOutput")

    def bcast_ap(t, n):
        return bass.AP(tensor=t, offset=0, ap=[[0, 128], [1, n]])

    with tile.TileContext(nc) as tc:
        with tc.tile_pool(name="consts", bufs=1) as consts, \
             tc.tile_pool(name="wpool", bufs=1) as wpool, \
             tc.tile_pool(name="acts", bufs=1) as acts, \
             tc.tile_pool(name="stage", bufs=2) as stage, \
             tc.tile_pool(name="prpool", bufs=4) as prpool, \
             tc.tile_pool(name="ppool", bufs=2, space="PSUM") as ppool, \
             tc.tile_pool(name="papool", bufs=2, space="PSUM") as papool, \
             tc.tile_pool(name="pmisc", bufs=2, space="PSUM") as pmisc:

            # ---- constants ----
            ident = consts.tile([128, 128], BF16)
            make_identity(nc, ident[:])
            bg_t = consts.tile([64, H], F32)
            nc.sync.dma_start(bg_t[:], bg.ap().rearrange("(h p) -> p h", p=64))
            bo_b = consts.tile([128, D], F32)
            nc.sync.dma_start(bo_b[:], bcast_ap(bo, D))
            gam_b = consts.tile([128, D], F32)
            nc.sync.dma_start(gam_b[:], bcast_ap(gamma, D))
            bet_b = consts.tile([128, D], F32)
            nc.sync.dma_start(bet_b[:], bcast_ap(beta, D))
            eps_t = consts.tile([128, 1], F32)
            nc.vector.memset(eps_t[:], EPS)

            # ---- weights: load fp32 in 128-row chunks, cast to bf16.
            #      Weight DMAs ride the sync queue; x DMAs ride the scalar
            #      queue so the two streams overlap. ----
            w_bf = {}
            for name, t in (("Wk", Wk), ("Wq", Wq), ("Wv", Wv), ("Wg", Wg)):
                wb = wpool.tile([128, 4, D], BF16, tag=f"w_{name}")
                for kc in range(4):
                    ws = stage.tile([128, D], F32, tag="wstage", bufs=4)
                    nc.sync.dma_start(ws[:], t[kc * 128:(kc + 1) * 128, :])
                    nc.vector.tensor_copy(wb[:, kc, :], ws[:])
                w_bf[name] = wb
            wo_b = wpool.tile([64, H, D], BF16)
            for h in range(H):
                ws = stage.tile([128, D], F32, tag="wostage", bufs=2)
                nc.sync.dma_start(ws[0:64, :], Wo[h * 64:(h + 1) * 64, :])
                nc.vector.tensor_copy(wo_b[:, h, :], ws[0:64, :])
            nbg = consts.tile([128, 4], F32)
            nc.sync.dma_start(nbg[:], bg.ap().rearrange("(m p) -> p m", p=128))
            nc.vector.tensor_scalar_mul(nbg[:], nbg[:], -1.0)

            # ---- tensors for x / projections ----
            xT = acts.tile([128, 4, N], BF16)
            sigT = acts.tile([64, H, NH], BF16)
            qT = acts.tile([128, 4, NH], BF16)
            kT = acts.tile([128, 4, N], BF16)
            v3 = acts.tile([128, NJT, H, DH + 1], BF16)
            nc.vector.memset(v3[:, :, :, DH:DH + 1], 1.0)

            def gates_unit(m, ic):
                # sigmoid(g+bg) = 1/(1+exp(-(g+bg))) -- uses the Exp table so
                # these can interleave freely with the attention exps
                def emit():
                    pm = pmisc.tile([128, 512], F32, tag="m")
                    for kc in range(4):
                        nc.tensor.matmul(pm[:], w_bf["Wg"][:, kc, m * 128:(m + 1) * 128],
                                         xT[:, kc, ic * 512:(ic + 1) * 512],
                                         start=(kc == 0), stop=(kc == 3))
                    e = stage.tile([128, 512], F32, tag="gexp")
                    nc.scalar.activation(e[:], pm[:], AF.Exp, scale=-1.0,
                                         bias=nbg[:, m:m + 1])
                    nc.vector.tensor_scalar_add(e[:], e[:], 1.0)
                    sp = stage.tile([128, 512], F32, tag="gsig")
                    nc.vector.reciprocal(sp[:], e[:])
                    nc.vector.tensor_copy(sigT[:, 2 * m, ic * 512:(ic + 1) * 512],
                                          sp[0:64, :])
                    nc.vector.tensor_copy(sigT[:, 2 * m + 1, ic * 512:(ic + 1) * 512],
                                          sp[64:128, :])
                return emit

            def qt_unit(m, ic):
                def emit():
                    pm = pmisc.tile([128, 512], F32, tag="m")
                    for kc in range(4):
                        nc.tensor.matmul(pm[:], w_bf["Wq"][:, kc, m * 128:(m + 1) * 128],
                                         xT[:, kc, ic * 512:(ic + 1) * 512],
                                         start=(kc == 0), stop=(kc == 3))
                    nc.vector.tensor_copy(qT[:, m, ic * 512:(ic + 1) * 512], pm[:])
                return emit

            def kt_unit(m, ic):
                def emit():
                    pm = pmisc.tile([128, 512], F32, tag="m")
                    for kc in range(4):
                        nc.tensor.matmul(pm[:], w_bf["Wk"][:, kc, m * 128:(m + 1) * 128],
                                         xT[:, kc, ic * 512:(ic + 1) * 512],
                                         start=(kc == 0), stop=(kc == 3))
                    nc.vector.tensor_copy(kT[:, m, ic * 512:(ic + 1) * 512], pm[:])
                return emit

            def v_unit(jt):
                def emit():
                    pm = pmisc.tile([128, 512], F32, tag="m")
                    for kc in range(4):
                        nc.tensor.matmul(pm[:], xT[:, kc, jt * 128:(jt + 1) * 128],
                                         w_bf["Wv"][:, kc, :],
                                         start=(kc == 0), stop=(kc == 3))
                    nc.vector.tensor_copy(
                        v3[:, jt, :, 0:DH],
                        pm[:].rearrange("p (h d) -> p h d", h=H))
                return emit

            # ---- x: load, cast, transpose; prelude projection units are
            #      emitted as soon as the xT columns they read exist, so
            #      gates/q/k/v overlap the transpose pipeline and attention
            #      can start while the tail of x is still being transposed.
            #      All sigmoids stay before the first exp (one table switch).
            def x_unit(nt):
                def emit():
                    xs = stage.tile([128, D], F32, tag="xstage", bufs=4)
                    dma_eng = nc.scalar if nt % 2 == 0 else nc.gpsimd
                    dma_eng.dma_start(xs[:], xkv[nt * 128:(nt + 1) * 128, :])
                    xb = stage.tile([128, D], BF16, tag="xbf")
                    nc.vector.tensor_copy(xb[:], xs[:])
                    for kc in range(4):
                        pt = pmisc.tile([128, 128], BF16, tag="m")
                        nc.tensor.transpose(pt[:], xb[:, kc * 128:(kc + 1) * 128],
                                            ident[:])
                        nc.vector.tensor_copy(xT[:, kc, nt * 128:(nt + 1) * 128],
                                              pt[:])
                return emit

            # transpose only the rows pair-0 needs immediately; nt 8..15 are
            # folded into pair-0's attention loop below
            prelude = {
                3: [kt_unit(0, 0)],
                4: [v_unit(0)],
                5: [qt_unit(0, 0), v_unit(1)],
                7: [kt_unit(0, 1), qt_unit(0, 1)],
            }
            for nt in range(8):
                x_unit(nt)()
                for unit in prelude.get(nt, []):
                    unit()

            # during pair p's attention, emit projections for pair p+1
            # (v3 for the remaining jt is finished inside pair-0 ic=0,
            # pipelined two key-tiles ahead of its consumer)
            queues = {
                0: [gates_unit(1, 0), gates_unit(1, 1)]
                   + [qt_unit(1, ic) for ic in range(2)]
                   + [kt_unit(1, ic) for ic in range(4)],
                1: [gates_unit(2, 0), gates_unit(2, 1)]
                   + [qt_unit(2, ic) for ic in range(2)]
                   + [kt_unit(2, ic) for ic in range(4)],
                2: [gates_unit(3, 0), gates_unit(3, 1)]
                   + [qt_unit(3, ic) for ic in range(2)]
                   + [kt_unit(3, ic) for ic in range(4)],
                3: None,  # filled per-ic below: Wo/LN for it 0..3 during ic=1
            }

            # ---- attention, per head pair ----
            gatedT = acts.tile([64, H, NH], BF16)

            def wo_unit(it, psum="m"):
                def emit():
                    xres = stage.tile([128, D], F32, tag=f"xres{it % 4}")
                    nc.scalar.dma_start(xres[:], xkv[it * 128:(it + 1) * 128, :])
                    if psum == "att":
                        pw = papool.tile([128, 512], F32, tag="att")
                    elif psum == "pd":
                        pw_full = ppool.tile([128, 1024], F32, tag="pd")
                        pw = pw_full[:, 0:512]
                    else:
                        pw = pmisc.tile([128, 512], F32, tag="m")
                    for h in range(H):
                        nc.tensor.matmul(pw[:], gatedT[:, h, it * 128:(it + 1) * 128],
                                         wo_b[:, h, :], start=(h == 0),
                                         stop=(h == H - 1))
                    y = stage.tile([128, D], F32, tag="y")
                    nc.vector.tensor_add(y[:], pw[:], xres[:])
                    if not trivial_bo:
                        nc.vector.tensor_add(y[:], y[:], bo_b[:])
                    st = stage.tile([128, 6], F32, tag="st")
                    nc.vector.bn_stats(st[:], y[:])
                    mv = stage.tile([128, 2], F32, tag="mv")
                    nc.vector.bn_aggr(mv[:], st[:])
                    ve = stage.tile([128, 1], F32, tag="ve")
                    nc.vector.tensor_add(ve[:], mv[:, 1:2], eps_t[:])
                    nc.vector.reciprocal(ve[:], ve[:])
                    nc.scalar.activation(ve[:], ve[:], AF.Sqrt)
                    z = stage.tile([128, D], F32, tag="z")
                    nc.vector.tensor_scalar(z[:], y[:], mv[:, 0:1], ve[:],
                                            OP.subtract, OP.mult)
                    if not trivial_gb:
                        nc.vector.tensor_mul(z[:], z[:], gam_b[:])
                        nc.vector.tensor_add(z[:], z[:], bet_b[:])
                    nc.sync.dma_start(out[it * 128:(it + 1) * 128, :], z[:])
                return emit

            for p in range(4):
                work = queues[p] or []
                wi = 0
                for ic in range(NH // 512):
                    if p == 3 and ic == 1:
                        work = [wo_unit(it) for it in range(4)]
                        wi = 0
                    pe_ = papool.tile([128, 512], F32, tag="att")
                    po_ = papool.tile([128, 512], F32, tag="att")

                    def dots_step(jt):
                        pd = ppool.tile([128, 1024], F32)
                        nc.tensor.matmul(pd[:, 0:512],
                                         kT[0:64, p, jt * 128:(jt + 1) * 128],
                                         qT[0:64, p, ic * 512:(ic + 1) * 512],
                                         start=True, stop=True,
                                         tile_position=(0, 0))
                        nc.tensor.matmul(pd[:, 512:1024],
                                         kT[64:128, p, jt * 128:(jt + 1) * 128],
                                         qT[64:128, p, ic * 512:(ic + 1) * 512],
                                         start=True, stop=True,
                                         tile_position=(64, 0))
                        return pd

                    # software pipeline: dots for jt+1 issue on the PE before
                    # the attnVs of jt, which wait on the exp of jt
                    pd_cur = dots_step(0)
                    for jt in range(NJT):
                        pr = prpool.tile([128, 2, 512], BF16, tag="pr")
                        nc.scalar.activation(
                            pr[:], pd_cur[:].rearrange("p (h x) -> p h x", h=2),
                            AF.Exp, scale=SCALE)
                        if jt + 1 < NJT:
                            pd_cur = dots_step(jt + 1)
                        if p == 0 and ic == 0:
                            if jt < 8:
                                x_unit(8 + jt)()
                            if jt == 4:
                                kt_unit(0, 2)()
                            elif jt == 8:
                                kt_unit(0, 3)()
                            if jt + 2 < NJT:
                                v_unit(jt + 2)()
                            elif jt == NJT - 2:
                                gates_unit(0, 0)()
                            else:
                                gates_unit(0, 1)()
                        elif wi < len(work) and (jt % 2 == 0 or wi > len(work) - 3):
                            work[wi]()
                            wi += 1
                        nc.tensor.matmul(pe_[0:65, :], v3[:, jt, 2 * p, :],
                                         pr[:, 0, :],
                                         start=(jt == 0), stop=(jt == NJT - 1))
                        nc.tensor.matmul(po_[0:65, :], v3[:, jt, 2 * p + 1, :],
                                         pr[:, 1, :],
                                         start=(jt == 0), stop=(jt == NJT - 1))
                    for hh, ph in ((2 * p, pe_), (2 * p + 1, po_)):
                        # evacuate PSUM fast (frees the accumulator bank for
                        # the next ic), then gate from SBUF off-critical-path
                        raw = stage.tile([65, 512], F32, tag="praw", bufs=4)
                        nc.vector.tensor_copy(raw[:], ph[0:65, :])
                        r0 = stage.tile([1, 512], F32, tag="r0")
                        nc.vector.reciprocal(r0[:], raw[64:65, :])
                        rb = stage.tile([64, 512], F32, tag="rb")
                        nc.gpsimd.partition_broadcast(rb[:], r0[:])
                        tmp = stage.tile([64, 512], F32, tag="tmp")
                        nc.vector.tensor_mul(tmp[:], raw[0:64, :], rb[:])
                        nc.vector.tensor_mul(gatedT[:, hh, ic * 512:(ic + 1) * 512],
                                             tmp[:], sigT[:, hh, ic * 512:(ic + 1) * 512])

            # ---- remaining Wo + LayerNorm tail units (it 4..7; 0..3 were
            #      interleaved into pair-3 attention). Three PSUM slots
            #      (pmisc/papool/ppool) keep the it-tiles pipelined. ----
            for it, ps in ((4, "m"), (5, "att"), (6, "pd"), (7, "m")):
                wo_unit(it, psum=ps)()

    nc.compile()
    return nc


_NC_CACHE = {}


def _get_nc(trivial_bo=False, trivial_gb=False):
    key = (trivial_bo, trivial_gb)
    if key not in _NC_CACHE:
        _NC_CACHE[key] = build_nc(*key)
    return _NC_CACHE[key]


def kernel(**inputs) -> np.ndarray:
    x = np.asarray(inputs["x"], dtype=np.float32)
    Wq = np.ascontiguousarray(np.asarray(inputs["Wq"], dtype=np.float32))
    Wkv = np.asarray(inputs["Wkv"], dtype=np.float32)
    Wk = np.ascontiguousarray(Wkv[:, :D])
    Wv = np.ascontiguousarray(Wkv[:, D:])
    Wg = np.ascontiguousarray(np.asarray(inputs["Wg"], dtype=np.float32))
    Wo = np.ascontiguousarray(np.asarray(inputs["Wo"], dtype=np.float32))
    bg = np.ascontiguousarray(np.asarray(inputs["bg"], dtype=np.float32))
    bo = np.ascontiguousarray(np.asarray(inputs["bo"], dtype=np.float32))
    gamma = np.ascontiguousarray(np.asarray(inputs["gamma"], dtype=np.float32))
    beta = np.ascontiguousarray(np.asarray(inputs["beta"], dtype=np.float32))

    trivial_bo = bool(np.all(bo == 0.0))
    trivial_gb = bool(np.all(gamma == 1.0) and np.all(beta == 0.0))
    nc = _get_nc(trivial_bo, trivial_gb)
    in_maps = []
    for c in range(NCORES):
        b, half = c // 2, c % 2
        rolled = np.ascontiguousarray(np.roll(x[b], -half * NH, axis=0))
        in_maps.append({"xkv": rolled, "Wq": Wq, "Wk": Wk, "Wv": Wv,
                        "Wg": Wg, "Wo": Wo, "bg": bg, "bo": bo,
                        "gamma": gamma, "beta": beta})
    res = run_bass_kernel_spmd(nc, in_maps, core_ids=list(range(NCORES)))
    out = np.empty((B, N, D), dtype=np.float32)
    for c in range(NCORES):
        b, half = c // 2, c % 2
        out[b, half * NH:(half + 1) * NH] = res.results[c]["out"]
    return out


# revision 42
# speedup vs baseline: 1.0759x; 1.0006x over previous
"""Trainium2 (8 NeuronCores) kernel for a gated-attention transformer block.

Reference computation (per batch b):
    q = x@Wq, [k|v] = x@Wkv, heads=8, dh=64
    attn = softmax(q k^T / 8) v
    out  = (attn * sigmoid(x@Wg + bg)) @ Wo + bo + x
    out  = LayerNorm(out) * gamma + beta

Sharding: 8 cores = 4 batches x 2 sequence-halves. Each core computes
k/v for its full batch (duplicated across the half-pair; avoids any
collective) and q/gates/output for its own 1024 rows. Row order of
keys/values is irrelevant to attention, so each core receives x[b]
rolled so its own rows come first; compile-time indices are then
identical across cores (SPMD-safe).

On-chip layout: activations transposed ([feature, seq]) for projections
and attention; dots computed as dotsT[j, i] with 2x row-tiled matmuls
(K=64 head pairs on PE quadrants), softmax denominator via a ones-column
augmented attn@v matmul (M=65), gating + denominator applied in
transposed layout, final Wo projection back to natural layout for the
residual + LayerNorm tail. All matmuls bf16 with fp32 PSUM accumulation.

Scheduling: projections for head-pair p+1 are emitted interleaved with
attention of pair p so the TensorEngine stays busy while the ScalarEngine
runs the (bottleneck) softmax exponentials. All sigmoids are emitted
before the first exp and the LayerNorm sqrts after the last one, so the
ScalarEngine's activation table is switched exactly twice.
"""

import sys
import os
import numpy as np

for _p in ("/opt/trn_rl_repo", "/root/.axon_site/_ro/trn_rl_repo"):
    if os.path.isdir(_p) and _p not in sys.path:
        sys.path.insert(0, _p)

import concourse.bass as bass
import concourse.tile as tile
from concourse import bacc, mybir
from concourse.bass_utils import run_bass_kernel_spmd
from concourse.masks import make_identity

F32 = mybir.dt.float32
BF16 = mybir.dt.bfloat16
AF = mybir.ActivationFunctionType
OP = mybir.AluOpType

B, N, D, H, DH = 4, 2048, 512, 8, 64
NH = N // 2          # rows owned per core
NJT = N // 128       # 16 key tiles
SCALE = DH ** -0.5   # 0.125
EPS = 1e-5
NCORES = 8


def build_nc(trivial_bo=False, trivial_gb=False):
    nc = bacc.Bacc("TRN2", target_bir_lowering=False, debug=False,
                   num_devices=NCORES)

    xkv = nc.dram_tensor("xkv", [N, D], F32, kind="ExternalInput")
    Wq = nc.dram_tensor("Wq", [D, D], F32, kind="ExternalInput")
    Wk = nc.dram_tensor("Wk", [D, D], F32, kind="ExternalInput")
    Wv = nc.dram_tensor("Wv", [D, D], F32, kind="ExternalInput")
    Wg = nc.dram_tensor("Wg", [D, D], F32, kind="ExternalInput")
    Wo = nc.dram_tensor("Wo", [D, D], F32, kind="ExternalInput")
    bg = nc.dram_tensor("bg", [D], F32, kind="ExternalInput")
    bo = nc.dram_tensor("bo", [D], F32, kind="ExternalInput")
    gamma = nc.dram_tensor("gamma", [D], F32, kind="ExternalInput")
    beta = nc.dram_tensor("beta", [D], F32, kind="ExternalInput")
    out = nc.dram_tensor("out", [NH, D], F32, kind="ExternalOutput")

    def bcast_ap(t, n):
        return bass.AP(tensor=t, offset=0, ap=[[0, 128], [1, n]])

    with tile.TileContext(nc) as tc:
        with tc.tile_pool(name="consts", bufs=1) as consts, \
             tc.tile_pool(name="wpool", bufs=1) as wpool, \
             tc.tile_pool(name="acts", bufs=1) as acts, \
             tc.tile_pool(name="stage", bufs=2) as stage, \
             tc.tile_pool(name="prpool", bufs=6) as prpool, \
             tc.tile_pool(name="ppool", bufs=2, space="PSUM") as ppool, \
             tc.tile_pool(name="papool", bufs=2, space="PSUM") as papool, \
             tc.tile_pool(name="pmisc", bufs=2, space="PSUM") as pmisc:

            # ---- constants ----
            ident = consts.tile([128, 128], BF16)
            make_identity(nc, ident[:])
            bg_t = consts.tile([64, H], F32)
            nc.sync.dma_start(bg_t[:], bg.ap().rearrange("(h p) -> p h", p=64))
            bo_b = consts.tile([128, D], F32)
            nc.sync.dma_start(bo_b[:], bcast_ap(bo, D))
            gam_b = consts.tile([128, D], F32)
            nc.sync.dma_start(gam_b[:], bcast_ap(gamma, D))
            bet_b = consts.tile([128, D], F32)
            nc.sync.dma_start(bet_b[:], bcast_ap(beta, D))
            eps_t = consts.tile([128, 1], F32)
            nc.vector.memset(eps_t[:], EPS)

            # ---- weights: load fp32 in 128-row chunks, cast to bf16.
            #      Weight DMAs ride the sync queue; x DMAs ride the scalar
            #      queue so the two streams overlap. ----
            w_bf = {}
            for name, t in (("Wk", Wk), ("Wq", Wq), ("Wv", Wv), ("Wg", Wg)):
                wb = wpool.tile([128, 4, D], BF16, tag=f"w_{name}")
                for kc in range(4):
                    ws = stage.tile([128, D], F32, tag="wstage", bufs=4)
                    nc.sync.dma_start(ws[:], t[kc * 128:(kc + 1) * 128, :])
                    nc.vector.tensor_copy(wb[:, kc, :], ws[:])
                w_bf[name] = wb
            wo_b = wpool.tile([64, H, D], BF16)
            for h in range(H):
                ws = stage.tile([128, D], F32, tag="wostage", bufs=2)
                nc.sync.dma_start(ws[0:64, :], Wo[h * 64:(h + 1) * 64, :])
                nc.vector.tensor_copy(wo_b[:, h, :], ws[0:64, :])
            nbg = consts.tile([128, 4], F32)
            nc.sync.dma_start(nbg[:], bg.ap().rearrange("(m p) -> p m", p=128))
            nc.vector.tensor_scalar_mul(nbg[:], nbg[:], -1.0)

            # ---- tensors for x / projections ----
            xT = acts.tile([128, 4, N], BF16)
            sigT = acts.tile([64, H, NH], BF16)
            qT = acts.tile([128, 4, NH], BF16)
            kT = acts.tile([128, 4, N], BF16)
            v3 = acts.tile([128, NJT, H, DH + 1], BF16)
            nc.vector.memset(v3[:, :, :, DH:DH + 1], 1.0)

            def gates_unit(m, ic):
                # sigmoid(g+bg) = 1/(1+exp(-(g+bg))) -- uses the Exp table so
                # these can interleave freely with the attention exps
                def emit():
                    pm = pmisc.tile([128, 512], F32, tag="m")
                    for kc in range(4):
                        nc.tensor.matmul(pm[:], w_bf["Wg"][:, kc, m * 128:(m + 1) * 128],
                                         xT[:, kc, ic * 512:(ic + 1) * 512],
                                         start=(kc == 0), stop=(kc == 3))
                    e = stage.tile([128, 512], F32, tag="gexp")
                    nc.scalar.activation(e[:], pm[:], AF.Exp, scale=-1.0,
                                         bias=nbg[:, m:m + 1])
                    nc.vector.tensor_scalar_add(e[:], e[:], 1.0)
                    sp = stage.tile([128, 512], F32, tag="gsig")
                    nc.vector.reciprocal(sp[:], e[:])
                    nc.vector.tensor_copy(sigT[:, 2 * m, ic * 512:(ic + 1) * 512],
                                          sp[0:64, :])
                    nc.vector.tensor_copy(sigT[:, 2 * m + 1, ic * 512:(ic + 1) * 512],
                                          sp[64:128, :])
                return emit

            def qt_unit(m, ic):
                def emit():
                    pm = pmisc.tile([128, 512], F32, tag="m")
                    for kc in range(4):
                        nc.tensor.matmul(pm[:], w_bf["Wq"][:, kc, m * 128:(m + 1) * 128],
                                         xT[:, kc, ic * 512:(ic + 1) * 512],
                                         start=(kc == 0), stop=(kc == 3))
                    nc.vector.tensor_copy(qT[:, m, ic * 512:(ic + 1) * 512], pm[:])
                return emit

            def kt_unit(m, ic):
                def emit():
                    pm = pmisc.tile([128, 512], F32, tag="m")
                    for kc in range(4):
                        nc.tensor.matmul(pm[:], w_bf["Wk"][:, kc, m * 128:(m + 1) * 128],
                                         xT[:, kc, ic * 512:(ic + 1) * 512],
                                         start=(kc == 0), stop=(kc == 3))
                    nc.vector.tensor_copy(kT[:, m, ic * 512:(ic + 1) * 512], pm[:])
                return emit

            def v_unit(jt):
                def emit():
                    pm = pmisc.tile([128, 512], F32, tag="m")
                    for kc in range(4):
                        nc.tensor.matmul(pm[:], xT[:, kc, jt * 128:(jt + 1) * 128],
                                         w_bf["Wv"][:, kc, :],
                                         start=(kc == 0), stop=(kc == 3))
                    nc.vector.tensor_copy(
                        v3[:, jt, :, 0:DH],
                        pm[:].rearrange("p (h d) -> p h d", h=H))
                return emit

            # ---- x: load, cast, transpose; prelude projection units are
            #      emitted as soon as the xT columns they read exist, so
            #      gates/q/k/v overlap the transpose pipeline and attention
            #      can start while the tail of x is still being transposed.
            #      All sigmoids stay before the first exp (one table switch).
            def x_unit(nt):
                def emit():
                    xs = stage.tile([128, D], F32, tag="xstage", bufs=4)
                    dma_eng = nc.scalar if nt % 2 == 0 else nc.gpsimd
                    dma_eng.dma_start(xs[:], xkv[nt * 128:(nt + 1) * 128, :])
                    xb = stage.tile([128, D], BF16, tag="xbf")
                    nc.vector.tensor_copy(xb[:], xs[:])
                    for kc in range(4):
                        pt = pmisc.tile([128, 128], BF16, tag="m")
                        nc.tensor.transpose(pt[:], xb[:, kc * 128:(kc + 1) * 128],
                                            ident[:])
                        nc.vector.tensor_copy(xT[:, kc, nt * 128:(nt + 1) * 128],
                                              pt[:])
                return emit

            # transpose only the rows pair-0 needs immediately; nt 8..15 are
            # folded into pair-0's attention loop below
            prelude = {
                3: [kt_unit(0, 0)],
                4: [v_unit(0)],
                5: [qt_unit(0, 0), v_unit(1)],
                7: [kt_unit(0, 1), qt_unit(0, 1)],
            }
            for nt in range(8):
                x_unit(nt)()
                for unit in prelude.get(nt, []):
                    unit()

            # during pair p's attention, emit projections for pair p+1
            # (v3 for the remaining jt is finished inside pair-0 ic=0,
            # pipelined two key-tiles ahead of its consumer)
            queues = {
                0: [gates_unit(1, 0), gates_unit(1, 1)]
                   + [qt_unit(1, ic) for ic in range(2)]
                   + [kt_unit(1, ic) for ic in range(4)],
                1: [gates_unit(2, 0), gates_unit(2, 1)]
                   + [qt_unit(2, ic) for ic in range(2)]
                   + [kt_unit(2, ic) for ic in range(4)],
                2: [gates_unit(3, 0), gates_unit(3, 1)]
                   + [qt_unit(3, ic) for ic in range(2)]
                   + [kt_unit(3, ic) for ic in range(4)],
                3: None,  # filled per-ic below: Wo/LN for it 0..3 during ic=1
            }

            # ---- attention, per head pair ----
            gatedT = acts.tile([64, H, NH], BF16)

            def wo_unit(it, psum="m"):
                def emit():
                    xres = stage.tile([128, D], F32, tag=f"xres{it % 4}")
                    nc.scalar.dma_start(xres[:], xkv[it * 128:(it + 1) * 128, :])
                    if psum == "att":
                        pw = papool.tile([128, 512], F32, tag="att")
                    elif psum == "pd":
                        pw_full = ppool.tile([128, 1024], F32, tag="pd")
                        pw = pw_full[:, 0:512]
                    else:
                        pw = pmisc.tile([128, 512], F32, tag="m")
                    for h in range(H):
                        nc.tensor.matmul(pw[:], gatedT[:, h, it * 128:(it + 1) * 128],
                                         wo_b[:, h, :], start=(h == 0),
                                         stop=(h == H - 1))
                    y = stage.tile([128, D], F32, tag="y")
                    nc.vector.tensor_add(y[:], pw[:], xres[:])
                    if not trivial_bo:
                        nc.vector.tensor_add(y[:], y[:], bo_b[:])
                    st = stage.tile([128, 6], F32, tag="st")
                    nc.vector.bn_stats(st[:], y[:])
                    mv = stage.tile([128, 2], F32, tag="mv")
                    nc.vector.bn_aggr(mv[:], st[:])
                    ve = stage.tile([128, 1], F32, tag="ve")
                    nc.vector.tensor_add(ve[:], mv[:, 1:2], eps_t[:])
                    nc.vector.reciprocal(ve[:], ve[:])
                    nc.scalar.activation(ve[:], ve[:], AF.Sqrt)
                    z = stage.tile([128, D], F32, tag="z")
                    nc.vector.tensor_scalar(z[:], y[:], mv[:, 0:1], ve[:],
                                            OP.subtract, OP.mult)
                    if not trivial_gb:
                        nc.vector.tensor_mul(z[:], z[:], gam_b[:])
                        nc.vector.tensor_add(z[:], z[:], bet_b[:])
                    nc.sync.dma_start(out[it * 128:(it + 1) * 128, :], z[:])
                return emit

            for p in range(4):
                work = queues[p] or []
                wi = 0
                for ic in range(NH // 512):
                    if p == 3 and ic == 1:
                        work = [wo_unit(it) for it in range(4)]
                        wi = 0
                    pe_ = papool.tile([128, 512], F32, tag="att")
                    po_ = papool.tile([128, 512], F32, tag="att")

                    def dots_step(jt):
                        pd = ppool.tile([128, 1024], F32)
                        nc.tensor.matmul(pd[:, 0:512],
                                         kT[0:64, p, jt * 128:(jt + 1) * 128],
                                         qT[0:64, p, ic * 512:(ic + 1) * 512],
                                         start=True, stop=True,
                                         tile_position=(0, 0))
                        nc.tensor.matmul(pd[:, 512:1024],
                                         kT[64:128, p, jt * 128:(jt + 1) * 128],
                                         qT[64:128, p, ic * 512:(ic + 1) * 512],
                                         start=True, stop=True,
                                         tile_position=(64, 0))
                        return pd

                    # software pipeline: dots for jt+1 issue on the PE before
                    # the attnVs of jt, which wait on the exp of jt
                    pd_cur = dots_step(0)
                    for jt in range(NJT):
                        pr = prpool.tile([128, 2, 512], BF16, tag="pr")
                        nc.scalar.activation(
                            pr[:], pd_cur[:].rearrange("p (h x) -> p h x", h=2),
                            AF.Exp, scale=SCALE)
                        if jt + 1 < NJT:
                            pd_cur = dots_step(jt + 1)
                        if p == 0 and ic == 0:
                            if jt < 8:
                                x_unit(8 + jt)()
                            if jt == 4:
                                kt_unit(0, 2)()
                            elif jt == 8:
                                kt_unit(0, 3)()
                            if jt + 2 < NJT:
                                v_unit(jt + 2)()
                            elif jt == NJT - 2:
                                gates_unit(0, 0)()
                            else:
                                gates_unit(0, 1)()
                        elif wi < len(work) and (jt % 2 == 0 or wi > len(work) - 3):
                            work[wi]()
                            wi += 1
                        nc.tensor.matmul(pe_[0:65, :], v3[:, jt, 2 * p, :],
                                         pr[:, 0, :],
                                         start=(jt == 0), stop=(jt == NJT - 1))
                        nc.tensor.matmul(po_[0:65, :], v3[:, jt, 2 * p + 1, :],
                                         pr[:, 1, :],
                                         start=(jt == 0), stop=(jt == NJT - 1))
                    for hh, ph in ((2 * p, pe_), (2 * p + 1, po_)):
                        # evacuate PSUM fast (frees the accumulator bank for
                        # the next ic), then gate from SBUF off-critical-path
                        raw = stage.tile([65, 512], F32, tag="praw", bufs=4)
                        nc.vector.tensor_copy(raw[:], ph[0:65, :])
                        r0 = stage.tile([1, 512], F32, tag="r0")
                        nc.vector.reciprocal(r0[:], raw[64:65, :])
                        rb = stage.tile([64, 512], F32, tag="rb")
                        nc.gpsimd.partition_broadcast(rb[:], r0[:])
                        tmp = stage.tile([64, 512], F32, tag="tmp")
                        nc.vector.tensor_mul(tmp[:], raw[0:64, :], rb[:])
                        nc.vector.tensor_mul(gatedT[:, hh, ic * 512:(ic + 1) * 512],
                                             tmp[:], sigT[:, hh, ic * 512:(ic + 1) * 512])

            # ---- remaining Wo + LayerNorm tail units (it 4..7; 0..3 were
            #      interleaved into pair-3 attention). Three PSUM slots
            #      (pmisc/papool/ppool) keep the it-tiles pipelined. ----
            for it, ps in ((4, "m"), (5, "att"), (6, "pd"), (7, "m")):
                wo_unit(it, psum=ps)()

    nc.compile()
    return nc


_NC_CACHE = {}


def _get_nc(trivial_bo=False, trivial_gb=False):
    key = (trivial_bo, trivial_gb)
    if key not in _NC_CACHE:
        _NC_CACHE[key] = build_nc(*key)
    return _NC_CACHE[key]


def kernel(**inputs) -> np.ndarray:
    x = np.asarray(inputs["x"], dtype=np.float32)
    Wq = np.ascontiguousarray(np.asarray(inputs["Wq"], dtype=np.float32))
    Wkv = np.asarray(inputs["Wkv"], dtype=np.float32)
    Wk = np.ascontiguousarray(Wkv[:, :D])
    Wv = np.ascontiguousarray(Wkv[:, D:])
    Wg = np.ascontiguousarray(np.asarray(inputs["Wg"], dtype=np.float32))
    Wo = np.ascontiguousarray(np.asarray(inputs["Wo"], dtype=np.float32))
    bg = np.ascontiguousarray(np.asarray(inputs["bg"], dtype=np.float32))
    bo = np.ascontiguousarray(np.asarray(inputs["bo"], dtype=np.float32))
    gamma = np.ascontiguousarray(np.asarray(inputs["gamma"], dtype=np.float32))
    beta = np.ascontiguousarray(np.asarray(inputs["beta"], dtype=np.float32))

    trivial_bo = bool(np.all(bo == 0.0))
    trivial_gb = bool(np.all(gamma == 1.0) and np.all(beta == 0.0))
    nc = _get_nc(trivial_bo, trivial_gb)
    in_maps = []
    for c in range(NCORES):
        b, half = c // 2, c % 2
        rolled = np.ascontiguousarray(np.roll(x[b], -half * NH, axis=0))
        in_maps.append({"xkv": rolled, "Wq": Wq, "Wk": Wk, "Wv": Wv,
                        "Wg": Wg, "Wo": Wo, "bg": bg, "bo": bo,
                        "gamma": gamma, "beta": beta})
    res = run_bass_kernel_spmd(nc, in_maps, core_ids=list(range(NCORES)))
    out = np.empty((B, N, D), dtype=np.float32)
    for c in range(NCORES):
        b, half = c // 2, c % 2
        out[b, half * NH:(half + 1) * NH] = res.results[c]["out"]
    return out


# revision 46
# speedup vs baseline: 1.0796x; 1.0035x over previous
"""Trainium2 (8 NeuronCores) kernel for a gated-attention transformer block.

Reference computation (per batch b):
    q = x@Wq, [k|v] = x@Wkv, heads=8, dh=64
    attn = softmax(q k^T / 8) v
    out  = (attn * sigmoid(x@Wg + bg)) @ Wo + bo + x
    out  = LayerNorm(out) * gamma + beta

Sharding: 8 cores = 4 batches x 2 sequence-halves. Each core computes
k/v for its full batch (duplicated across the half-pair; avoids any
collective) and q/gates/output for its own 1024 rows. Row order of
keys/values is irrelevant to attention, so each core receives x[b]
rolled so its own rows come first; compile-time indices are then
identical across cores (SPMD-safe).

On-chip layout: activations transposed ([feature, seq]) for projections
and attention; dots computed as dotsT[j, i] with 2x row-tiled matmuls
(K=64 head pairs on PE quadrants), softmax denominator via a ones-column
augmented attn@v matmul (M=65), gating + denominator applied in
transposed layout, final Wo projection back to natural layout for the
residual + LayerNorm tail. All matmuls bf16 with fp32 PSUM accumulation.

Scheduling: projections for head-pair p+1 are emitted interleaved with
attention of pair p so the TensorEngine stays busy while the ScalarEngine
runs the (bottleneck) softmax exponentials. All sigmoids are emitted
before the first exp and the LayerNorm sqrts after the last one, so the
ScalarEngine's activation table is switched exactly twice.
"""

import sys
import os
import numpy as np

for _p in ("/opt/trn_rl_repo", "/root/.axon_site/_ro/trn_rl_repo"):
    if os.path.isdir(_p) and _p not in sys.path:
        sys.path.insert(0, _p)

import concourse.bass as bass
import concourse.tile as tile
from concourse import bacc, mybir
from concourse.bass_utils import run_bass_kernel_spmd
from concourse.masks import make_identity

F32 = mybir.dt.float32
BF16 = mybir.dt.bfloat16
AF = mybir.ActivationFunctionType
OP = mybir.AluOpType

B, N, D, H, DH = 4, 2048, 512, 8, 64
NH = N // 2          # rows owned per core
NJT = N // 128       # 16 key tiles
SCALE = DH ** -0.5   # 0.125
EPS = 1e-5
NCORES = 8


def build_nc(trivial_bo=False, trivial_gb=False):
    nc = bacc.Bacc("TRN2", target_bir_lowering=False, debug=False,
                   num_devices=NCORES)

    xkv = nc.dram_tensor("xkv", [N, D], F32, kind="ExternalInput")
    Wq = nc.dram_tensor("Wq", [D, D], F32, kind="ExternalInput")
    Wk = nc.dram_tensor("Wk", [D, D], F32, kind="ExternalInput")
    Wv = nc.dram_tensor("Wv", [D, D], F32, kind="ExternalInput")
    Wg = nc.dram_tensor("Wg", [D, D], F32, kind="ExternalInput")
    Wo = nc.dram_tensor("Wo", [D, D], F32, kind="ExternalInput")
    bg = nc.dram_tensor("bg", [D], F32, kind="ExternalInput")
    bo = nc.dram_tensor("bo", [D], F32, kind="ExternalInput")
    gamma = nc.dram_tensor("gamma", [D], F32, kind="ExternalInput")
    beta = nc.dram_tensor("beta", [D], F32, kind="ExternalInput")
    out = nc.dram_tensor("out", [NH, D], F32, kind="ExternalOutput")

    def bcast_ap(t, n):
        return bass.AP(tensor=t, offset=0, ap=[[0, 128], [1, n]])

    with tile.TileContext(nc) as tc:
        with tc.tile_pool(name="consts", bufs=1) as consts, \
             tc.tile_pool(name="wpool", bufs=1) as wpool, \
             tc.tile_pool(name="acts", bufs=1) as acts, \
             tc.tile_pool(name="stage", bufs=2) as stage, \
             tc.tile_pool(name="prpool", bufs=6) as prpool, \
             tc.tile_pool(name="ppool", bufs=2, space="PSUM") as ppool, \
             tc.tile_pool(name="papool", bufs=2, space="PSUM") as papool, \
             tc.tile_pool(name="pmisc", bufs=2, space="PSUM") as pmisc:

            # ---- constants ----
            ident = consts.tile([128, 128], BF16)
            make_identity(nc, ident[:])
            bg_t = consts.tile([64, H], F32)
            nc.sync.dma_start(bg_t[:], bg.ap().rearrange("(h p) -> p h", p=64))
            bo_b = consts.tile([128, D], F32)
            nc.sync.dma_start(bo_b[:], bcast_ap(bo, D))
            gam_b = consts.tile([128, D], F32)
            nc.sync.dma_start(gam_b[:], bcast_ap(gamma, D))
            bet_b = consts.tile([128, D], F32)
            nc.sync.dma_start(bet_b[:], bcast_ap(beta, D))
            eps_t = consts.tile([128, 1], F32)
            nc.vector.memset(eps_t[:], EPS)

            # ---- weights: load fp32 in 128-row chunks, cast to bf16.
            #      Weight DMAs ride the sync queue; x DMAs ride the scalar
            #      queue so the two streams overlap. ----
            w_bf = {}
            for name, t in (("Wk", Wk), ("Wq", Wq), ("Wv", Wv), ("Wg", Wg)):
                wb = wpool.tile([128, 4, D], BF16, tag=f"w_{name}")
                for kc in range(4):
                    ws = stage.tile([128, D], F32, tag="wstage", bufs=4)
                    nc.sync.dma_start(ws[:], t[kc * 128:(kc + 1) * 128, :])
                    nc.vector.tensor_copy(wb[:, kc, :], ws[:])
                w_bf[name] = wb
            wo_b = wpool.tile([64, H, D], BF16)
            for h in range(H):
                ws = stage.tile([128, D], F32, tag="wostage", bufs=2)
                nc.sync.dma_start(ws[0:64, :], Wo[h * 64:(h + 1) * 64, :])
                nc.vector.tensor_copy(wo_b[:, h, :], ws[0:64, :])
            nbg = consts.tile([128, 4], F32)
            nc.sync.dma_start(nbg[:], bg.ap().rearrange("(m p) -> p m", p=128))
            nc.vector.tensor_scalar_mul(nbg[:], nbg[:], -1.0)

            # ---- tensors for x / projections ----
            xT = acts.tile([128, 4, N], BF16)
            sigT = acts.tile([64, H, NH], BF16)
            qT = acts.tile([128, 4, NH], BF16)
            kT = acts.tile([128, 4, N], BF16)
            v3 = acts.tile([128, NJT, H, DH + 1], BF16)
            nc.vector.memset(v3[:, :, :, DH:DH + 1], 1.0)

            def gates_unit(m, ic):
                # sigmoid(g+bg) = 1/(1+exp(-(g+bg))) -- uses the Exp table so
                # these can interleave freely with the attention exps
                def emit():
                    pm = pmisc.tile([128, 512], F32, tag="m")
                    for kc in range(4):
                        nc.tensor.matmul(pm[:], w_bf["Wg"][:, kc, m * 128:(m + 1) * 128],
                                         xT[:, kc, ic * 512:(ic + 1) * 512],
                                         start=(kc == 0), stop=(kc == 3))
                    e = stage.tile([128, 512], F32, tag="gexp")
                    nc.scalar.activation(e[:], pm[:], AF.Exp, scale=-1.0,
                                         bias=nbg[:, m:m + 1])
                    nc.vector.tensor_scalar_add(e[:], e[:], 1.0)
                    sp = stage.tile([128, 512], F32, tag="gsig")
                    nc.vector.reciprocal(sp[:], e[:])
                    nc.vector.tensor_copy(sigT[:, 2 * m, ic * 512:(ic + 1) * 512],
                                          sp[0:64, :])
                    nc.vector.tensor_copy(sigT[:, 2 * m + 1, ic * 512:(ic + 1) * 512],
                                          sp[64:128, :])
                return emit

            def qt_unit(m, ic):
                def emit():
                    pm = pmisc.tile([128, 512], F32, tag="m")
                    for kc in range(4):
                        nc.tensor.matmul(pm[:], w_bf["Wq"][:, kc, m * 128:(m + 1) * 128],
                                         xT[:, kc, ic * 512:(ic + 1) * 512],
                                         start=(kc == 0), stop=(kc == 3))
                    nc.vector.tensor_copy(qT[:, m, ic * 512:(ic + 1) * 512], pm[:])
                return emit

            def kt_unit(m, ic):
                def emit():
                    pm = pmisc.tile([128, 512], F32, tag="m")
                    for kc in range(4):
                        nc.tensor.matmul(pm[:], w_bf["Wk"][:, kc, m * 128:(m + 1) * 128],
                                         xT[:, kc, ic * 512:(ic + 1) * 512],
                                         start=(kc == 0), stop=(kc == 3))
                    nc.vector.tensor_copy(kT[:, m, ic * 512:(ic + 1) * 512], pm[:])
                return emit

            def v_unit(jt):
                def emit():
                    pm = pmisc.tile([128, 512], F32, tag="m")
                    for kc in range(4):
                        nc.tensor.matmul(pm[:], xT[:, kc, jt * 128:(jt + 1) * 128],
                                         w_bf["Wv"][:, kc, :],
                                         start=(kc == 0), stop=(kc == 3))
                    nc.vector.tensor_copy(
                        v3[:, jt, :, 0:DH],
                        pm[:].rearrange("p (h d) -> p h d", h=H))
                return emit

            # ---- x: load, cast, transpose; prelude projection units are
            #      emitted as soon as the xT columns they read exist, so
            #      gates/q/k/v overlap the transpose pipeline and attention
            #      can start while the tail of x is still being transposed.
            #      All sigmoids stay before the first exp (one table switch).
            def x_unit(nt):
                # two 128x128 transposes share one PSUM tile and one evac
                def emit():
                    xs = stage.tile([128, D], F32, tag="xstage", bufs=4)
                    dma_eng = nc.scalar if nt % 2 == 0 else nc.gpsimd
                    dma_eng.dma_start(xs[:], xkv[nt * 128:(nt + 1) * 128, :])
                    xb = stage.tile([128, D], BF16, tag="xbf")
                    nc.vector.tensor_copy(xb[:], xs[:])
                    for half in range(2):
                        pt = pmisc.tile([128, 2, 128], BF16, tag="m")
                        for j in range(2):
                            kc = 2 * half + j
                            nc.tensor.transpose(pt[:, j, :],
                                                xb[:, kc * 128:(kc + 1) * 128],
                                                ident[:])
                        nc.vector.tensor_copy(
                            xT[:, 2 * half:2 * half + 2, nt * 128:(nt + 1) * 128],
                            pt[:])
                return emit

            # transpose only the rows pair-0 needs immediately; nt 8..15 are
            # folded into pair-0's attention loop below
            prelude = {
                3: [kt_unit(0, 0)],
                4: [v_unit(0)],
                5: [qt_unit(0, 0), v_unit(1)],
                7: [kt_unit(0, 1), qt_unit(0, 1)],
            }
            for nt in range(8):
                x_unit(nt)()
                for unit in prelude.get(nt, []):
                    unit()

            # during pair p's attention, emit projections for pair p+1
            # (v3 for the remaining jt is finished inside pair-0 ic=0,
            # pipelined two key-tiles ahead of its consumer)
            queues = {
                0: [gates_unit(1, 0), gates_unit(1, 1)]
                   + [qt_unit(1, ic) for ic in range(2)]
                   + [kt_unit(1, ic) for ic in range(4)],
                1: [gates_unit(2, 0), gates_unit(2, 1)]
                   + [qt_unit(2, ic) for ic in range(2)]
                   + [kt_unit(2, ic) for ic in range(4)],
                2: [gates_unit(3, 0), gates_unit(3, 1)]
                   + [qt_unit(3, ic) for ic in range(2)]
                   + [kt_unit(3, ic) for ic in range(4)],
                3: None,  # filled per-ic below: Wo/LN for it 0..3 during ic=1
            }

            # ---- attention, per head pair ----
            gatedT = acts.tile([64, H, NH], BF16)

            def wo_unit(it, psum="m", act_ln=False):
                def emit():
                    xres = stage.tile([128, D], F32, tag=f"xres{it % 4}")
                    nc.scalar.dma_start(xres[:], xkv[it * 128:(it + 1) * 128, :])
                    if psum == "att":
                        pw = papool.tile([128, 512], F32, tag="att")
                    elif psum == "pd":
                        pw_full = ppool.tile([128, 1024], F32, tag="pd")
                        pw = pw_full[:, 0:512]
                    else:
                        pw = pmisc.tile([128, 512], F32, tag="m")
                    for h in range(H):
                        nc.tensor.matmul(pw[:], gatedT[:, h, it * 128:(it + 1) * 128],
                                         wo_b[:, h, :], start=(h == 0),
                                         stop=(h == H - 1))
                    y = stage.tile([128, D], F32, tag="y")
                    nc.vector.tensor_add(y[:], pw[:], xres[:])
                    if not trivial_bo:
                        nc.vector.tensor_add(y[:], y[:], bo_b[:])
                    ve = stage.tile([128, 1], F32, tag="ve")
                    if act_ln:
                        # LN statistics on the (tail-idle) ScalarEngine:
                        # accum_out gives per-row sum / sum-of-squares
                        cp = stage.tile([128, D], F32, tag="gexp")
                        sm = stage.tile([128, 2], F32, tag="mv")
                        nc.scalar.activation(cp[:], y[:], AF.Copy,
                                             accum_out=sm[:, 0:1])
                        nc.scalar.activation(cp[:], y[:], AF.Square,
                                             accum_out=sm[:, 1:2])
                        mu = stage.tile([128, 1], F32, tag="muT")
                        nc.vector.tensor_scalar_mul(mu[:], sm[:, 0:1], 1.0 / D)
                        m2 = stage.tile([128, 1], F32, tag="m2T")
                        nc.vector.tensor_mul(m2[:], mu[:], mu[:])
                        nc.vector.tensor_scalar_mul(ve[:], sm[:, 1:2], 1.0 / D)
                        nc.vector.tensor_sub(ve[:], ve[:], m2[:])
                        nc.vector.tensor_add(ve[:], ve[:], eps_t[:])
                        mu_ap = mu[:]
                    else:
                        st = stage.tile([128, 6], F32, tag="st")
                        nc.vector.bn_stats(st[:], y[:])
                        mv = stage.tile([128, 2], F32, tag="mv")
                        nc.vector.bn_aggr(mv[:], st[:])
                        nc.vector.tensor_add(ve[:], mv[:, 1:2], eps_t[:])
                        mu_ap = mv[:, 0:1]
                    nc.vector.reciprocal(ve[:], ve[:])
                    nc.scalar.activation(ve[:], ve[:], AF.Sqrt)
                    z = stage.tile([128, D], F32, tag="z")
                    nc.vector.tensor_scalar(z[:], y[:], mu_ap, ve[:],
                                            OP.subtract, OP.mult)
                    if not trivial_gb:
                        nc.vector.tensor_mul(z[:], z[:], gam_b[:])
                        nc.vector.tensor_add(z[:], z[:], bet_b[:])
                    nc.sync.dma_start(out[it * 128:(it + 1) * 128, :], z[:])
                return emit

            for p in range(4):
                work = queues[p] or []
                wi = 0
                for ic in range(NH // 512):
                    if p == 3 and ic == 1:
                        work = [wo_unit(it) for it in range(4)]
                        wi = 0
                    pe_ = papool.tile([128, 512], F32, tag="att")
                    po_ = papool.tile([128, 512], F32, tag="att")

                    def dots_step(jt):
                        pd = ppool.tile([128, 1024], F32)
                        nc.tensor.matmul(pd[:, 0:512],
                                         kT[0:64, p, jt * 128:(jt + 1) * 128],
                                         qT[0:64, p, ic * 512:(ic + 1) * 512],
                                         start=True, stop=True,
                                         tile_position=(0, 0))
                        nc.tensor.matmul(pd[:, 512:1024],
                                         kT[64:128, p, jt * 128:(jt + 1) * 128],
                                         qT[64:128, p, ic * 512:(ic + 1) * 512],
                                         start=True, stop=True,
                                         tile_position=(64, 0))
                        return pd

                    # software pipeline: dots for jt+1 issue on the PE before
                    # the attnVs of jt, which wait on the exp of jt
                    pd_cur = dots_step(0)
                    for jt in range(NJT):
                        pr = prpool.tile([128, 2, 512], BF16, tag="pr")
                        nc.scalar.activation(
                            pr[:], pd_cur[:].rearrange("p (h x) -> p h x", h=2),
                            AF.Exp, scale=SCALE)
                        if jt + 1 < NJT:
                            pd_cur = dots_step(jt + 1)
                        if p == 0 and ic == 0:
                            if jt < 8:
                                x_unit(8 + jt)()
                            if jt == 4:
                                kt_unit(0, 2)()
                            elif jt == 8:
                                kt_unit(0, 3)()
                            if jt + 2 < NJT:
                                v_unit(jt + 2)()
                            elif jt == NJT - 2:
                                gates_unit(0, 0)()
                            else:
                                gates_unit(0, 1)()
                        elif wi < len(work) and (jt % 2 == 0 or wi > len(work) - 3):
                            work[wi]()
                            wi += 1
                        nc.tensor.matmul(pe_[0:65, :], v3[:, jt, 2 * p, :],
                                         pr[:, 0, :],
                                         start=(jt == 0), stop=(jt == NJT - 1))
                        nc.tensor.matmul(po_[0:65, :], v3[:, jt, 2 * p + 1, :],
                                         pr[:, 1, :],
                                         start=(jt == 0), stop=(jt == NJT - 1))
                    for hh, ph in ((2 * p, pe_), (2 * p + 1, po_)):
                        # evacuate PSUM fast (frees the accumulator bank for
                        # the next ic), then gate from SBUF off-critical-path
                        raw = stage.tile([65, 512], F32, tag="praw", bufs=4)
                        nc.vector.tensor_copy(raw[:], ph[0:65, :])
                        r0 = stage.tile([1, 512], F32, tag="r0")
                        nc.vector.reciprocal(r0[:], raw[64:65, :])
                        rb = stage.tile([64, 512], F32, tag="rb")
                        nc.gpsimd.partition_broadcast(rb[:], r0[:])
                        tmp = stage.tile([64, 512], F32, tag="tmp")
                        nc.vector.tensor_mul(tmp[:], raw[0:64, :], rb[:])
                        nc.vector.tensor_mul(gatedT[:, hh, ic * 512:(ic + 1) * 512],
                                             tmp[:], sigT[:, hh, ic * 512:(ic + 1) * 512])

            # ---- remaining Wo + LayerNorm tail units (it 4..7; 0..3 were
            #      interleaved into pair-3 attention). Three PSUM slots
            #      (pmisc/papool/ppool) keep the it-tiles pipelined. ----
            for it, ps in ((4, "m"), (5, "att"), (6, "pd"), (7, "m")):
                wo_unit(it, psum=ps, act_ln=True)()

    nc.compile()
    return nc


_NC_CACHE = {}


def _get_nc(trivial_bo=False, trivial_gb=False):
    key = (trivial_bo, trivial_gb)
    if key not in _NC_CACHE:
        _NC_CACHE[key] = build_nc(*key)
    return _NC_CACHE[key]


def kernel(**inputs) -> np.ndarray:
    x = np.asarray(inputs["x"], dtype=np.float32)
    Wq = np.ascontiguousarray(np.asarray(inputs["Wq"], dtype=np.float32))
    Wkv = np.asarray(inputs["Wkv"], dtype=np.float32)
    Wk = np.ascontiguousarray(Wkv[:, :D])
    Wv = np.ascontiguousarray(Wkv[:, D:])
    Wg = np.ascontiguousarray(np.asarray(inputs["Wg"], dtype=np.float32))
    Wo = np.ascontiguousarray(np.asarray(inputs["Wo"], dtype=np.float32))
    bg = np.ascontiguousarray(np.asarray(inputs["bg"], dtype=np.float32))
    bo = np.ascontiguousarray(np.asarray(inputs["bo"], dtype=np.float32))
    gamma = np.ascontiguousarray(np.asarray(inputs["gamma"], dtype=np.float32))
    beta = np.ascontiguousarray(np.asarray(inputs["beta"], dtype=np.float32))

    trivial_bo = bool(np.all(bo == 0.0))
    trivial_gb = bool(np.all(gamma == 1.0) and np.all(beta == 0.0))
    nc = _get_nc(trivial_bo, trivial_gb)
    in_maps = []
    for c in range(NCORES):
        b, half = c // 2, c % 2
        rolled = np.ascontiguousarray(np.roll(x[b], -half * NH, axis=0))
        in_maps.append({"xkv": rolled, "Wq": Wq, "Wk": Wk, "Wv": Wv,
                        "Wg": Wg, "Wo": Wo, "bg": bg, "bo": bo,
                        "gamma": gamma, "beta": beta})
    res = run_bass_kernel_spmd(nc, in_maps, core_ids=list(range(NCORES)))
    out = np.empty((B, N, D), dtype=np.float32)
    for c in range(NCORES):
        b, half = c // 2, c % 2
        out[b, half * NH:(half + 1) * NH] = res.results[c]["out"]
    return out


# revision 48
# speedup vs baseline: 1.0902x; 1.0098x over previous
"""Trainium2 (8 NeuronCores) kernel for a gated-attention transformer block.

Reference computation (per batch b):
    q = x@Wq, [k|v] = x@Wkv, heads=8, dh=64
    attn = softmax(q k^T / 8) v
    out  = (attn * sigmoid(x@Wg + bg)) @ Wo + bo + x
    out  = LayerNorm(out) * gamma + beta

Sharding: 8 cores = 4 batches x 2 sequence-halves. Each core computes
k/v for its full batch (duplicated across the half-pair; avoids any
collective) and q/gates/output for its own 1024 rows. Row order of
keys/values is irrelevant to attention, so each core receives x[b]
rolled so its own rows come first; compile-time indices are then
identical across cores (SPMD-safe).

On-chip layout: activations transposed ([feature, seq]) for projections
and attention; dots computed as dotsT[j, i] with 2x row-tiled matmuls
(K=64 head pairs on PE quadrants), softmax denominator via a ones-column
augmented attn@v matmul (M=65), gating + denominator applied in
transposed layout, final Wo projection back to natural layout for the
residual + LayerNorm tail. All matmuls bf16 with fp32 PSUM accumulation.

Scheduling: projections for head-pair p+1 are emitted interleaved with
attention of pair p so the TensorEngine stays busy while the ScalarEngine
runs the (bottleneck) softmax exponentials. All sigmoids are emitted
before the first exp and the LayerNorm sqrts after the last one, so the
ScalarEngine's activation table is switched exactly twice.
"""

import sys
import os
import numpy as np

for _p in ("/opt/trn_rl_repo", "/root/.axon_site/_ro/trn_rl_repo"):
    if os.path.isdir(_p) and _p not in sys.path:
        sys.path.insert(0, _p)

import concourse.bass as bass
import concourse.tile as tile
from concourse import bacc, mybir
from concourse.bass_utils import run_bass_kernel_spmd
from concourse.masks import make_identity

F32 = mybir.dt.float32
BF16 = mybir.dt.bfloat16
AF = mybir.ActivationFunctionType
OP = mybir.AluOpType

B, N, D, H, DH = 4, 2048, 512, 8, 64
NH = N // 2          # rows owned per core
NJT = N // 128       # 16 key tiles
SCALE = DH ** -0.5   # 0.125
EPS = 1e-5
NCORES = 8


def build_nc(trivial_bo=False, trivial_gb=False):
    nc = bacc.Bacc("TRN2", target_bir_lowering=False, debug=False,
                   num_devices=NCORES)

    xkv = nc.dram_tensor("xkv", [N, D], F32, kind="ExternalInput")
    Wq = nc.dram_tensor("Wq", [D, D], F32, kind="ExternalInput")
    Wk = nc.dram_tensor("Wk", [D, D], F32, kind="ExternalInput")
    Wv = nc.dram_tensor("Wv", [D, D], F32, kind="ExternalInput")
    Wg = nc.dram_tensor("Wg", [D, D], F32, kind="ExternalInput")
    Wo = nc.dram_tensor("Wo", [D, D], F32, kind="ExternalInput")
    bg = nc.dram_tensor("bg", [D], F32, kind="ExternalInput")
    bo = nc.dram_tensor("bo", [D], F32, kind="ExternalInput")
    gamma = nc.dram_tensor("gamma", [D], F32, kind="ExternalInput")
    beta = nc.dram_tensor("beta", [D], F32, kind="ExternalInput")
    out = nc.dram_tensor("out", [NH, D], F32, kind="ExternalOutput")

    def bcast_ap(t, n):
        return bass.AP(tensor=t, offset=0, ap=[[0, 128], [1, n]])

    with tile.TileContext(nc) as tc:
        with tc.tile_pool(name="consts", bufs=1) as consts, \
             tc.tile_pool(name="wpool", bufs=1) as wpool, \
             tc.tile_pool(name="acts", bufs=1) as acts, \
             tc.tile_pool(name="stage", bufs=2) as stage, \
             tc.tile_pool(name="prpool", bufs=6) as prpool, \
             tc.tile_pool(name="ppool", bufs=2, space="PSUM") as ppool, \
             tc.tile_pool(name="papool", bufs=2, space="PSUM") as papool, \
             tc.tile_pool(name="pmisc", bufs=2, space="PSUM") as pmisc:

            # ---- constants ----
            ident = consts.tile([128, 128], BF16)
            make_identity(nc, ident[:])
            bg_t = consts.tile([64, H], F32)
            nc.sync.dma_start(bg_t[:], bg.ap().rearrange("(h p) -> p h", p=64))
            bo_b = consts.tile([128, D], F32)
            nc.sync.dma_start(bo_b[:], bcast_ap(bo, D))
            gam_b = consts.tile([128, D], F32)
            nc.sync.dma_start(gam_b[:], bcast_ap(gamma, D))
            bet_b = consts.tile([128, D], F32)
            nc.sync.dma_start(bet_b[:], bcast_ap(beta, D))
            eps_t = consts.tile([128, 1], F32)
            nc.vector.memset(eps_t[:], EPS)

            # ---- weights: load fp32 in 128-row chunks, cast to bf16.
            #      Weight DMAs ride the sync queue; x DMAs ride the scalar
            #      queue so the two streams overlap. ----
            w_bf = {}

            def load_weight(name, t):
                def emit():
                    wb = wpool.tile([128, 4, D], BF16, tag=f"w_{name}")
                    for kc in range(4):
                        ws = stage.tile([128, D], F32, tag="wstage", bufs=4)
                        nc.sync.dma_start(ws[:], t[kc * 128:(kc + 1) * 128, :])
                        nc.vector.tensor_copy(wb[:, kc, :], ws[:])
                    w_bf[name] = wb
                return emit

            # Only Wk is loaded before the x pipeline: its cast is the one
            # the first dots transitively waits on (DVE executes in order).
            load_weight("Wk", Wk)()
            nbg = consts.tile([128, 4], F32)
            nc.sync.dma_start(nbg[:], bg.ap().rearrange("(m p) -> p m", p=128))
            nc.vector.tensor_scalar_mul(nbg[:], nbg[:], -1.0)

            # ---- tensors for x / projections ----
            xT = acts.tile([128, 4, N], BF16)
            sigT = acts.tile([64, H, NH], BF16)
            qT = acts.tile([128, 4, NH], BF16)
            kT = acts.tile([128, 4, N], BF16)
            v3 = acts.tile([128, NJT, H, DH + 1], BF16)
            nc.vector.memset(v3[:, :, :, DH:DH + 1], 1.0)

            def gates_unit(m, ic):
                # sigmoid(g+bg) = 1/(1+exp(-(g+bg))) -- uses the Exp table so
                # these can interleave freely with the attention exps
                def emit():
                    pm = pmisc.tile([128, 512], F32, tag="m")
                    for kc in range(4):
                        nc.tensor.matmul(pm[:], w_bf["Wg"][:, kc, m * 128:(m + 1) * 128],
                                         xT[:, kc, ic * 512:(ic + 1) * 512],
                                         start=(kc == 0), stop=(kc == 3))
                    e = stage.tile([128, 512], F32, tag="gexp")
                    nc.scalar.activation(e[:], pm[:], AF.Exp, scale=-1.0,
                                         bias=nbg[:, m:m + 1])
                    nc.vector.tensor_scalar_add(e[:], e[:], 1.0)
                    sp = stage.tile([128, 512], F32, tag="gsig")
                    nc.vector.reciprocal(sp[:], e[:])
                    nc.vector.tensor_copy(sigT[:, 2 * m, ic * 512:(ic + 1) * 512],
                                          sp[0:64, :])
                    nc.vector.tensor_copy(sigT[:, 2 * m + 1, ic * 512:(ic + 1) * 512],
                                          sp[64:128, :])
                return emit

            def qt_unit(m, ic):
                def emit():
                    pm = pmisc.tile([128, 512], F32, tag="m")
                    for kc in range(4):
                        nc.tensor.matmul(pm[:], w_bf["Wq"][:, kc, m * 128:(m + 1) * 128],
                                         xT[:, kc, ic * 512:(ic + 1) * 512],
                                         start=(kc == 0), stop=(kc == 3))
                    nc.vector.tensor_copy(qT[:, m, ic * 512:(ic + 1) * 512], pm[:])
                return emit

            def kt_unit(m, ic):
                def emit():
                    pm = pmisc.tile([128, 512], F32, tag="m")
                    for kc in range(4):
                        nc.tensor.matmul(pm[:], w_bf["Wk"][:, kc, m * 128:(m + 1) * 128],
                                         xT[:, kc, ic * 512:(ic + 1) * 512],
                                         start=(kc == 0), stop=(kc == 3))
                    nc.vector.tensor_copy(kT[:, m, ic * 512:(ic + 1) * 512], pm[:])
                return emit

            def v_unit(jt):
                def emit():
                    pm = pmisc.tile([128, 512], F32, tag="m")
                    for kc in range(4):
                        nc.tensor.matmul(pm[:], xT[:, kc, jt * 128:(jt + 1) * 128],
                                         w_bf["Wv"][:, kc, :],
                                         start=(kc == 0), stop=(kc == 3))
                    nc.vector.tensor_copy(
                        v3[:, jt, :, 0:DH],
                        pm[:].rearrange("p (h d) -> p h d", h=H))
                return emit

            # ---- x: load, cast, transpose; prelude projection units are
            #      emitted as soon as the xT columns they read exist, so
            #      gates/q/k/v overlap the transpose pipeline and attention
            #      can start while the tail of x is still being transposed.
            #      All sigmoids stay before the first exp (one table switch).
            def x_unit(nt):
                # two 128x128 transposes share one PSUM tile and one evac
                def emit():
                    xs = stage.tile([128, D], F32, tag="xstage", bufs=4)
                    dma_eng = nc.scalar if nt % 2 == 0 else nc.gpsimd
                    dma_eng.dma_start(xs[:], xkv[nt * 128:(nt + 1) * 128, :])
                    xb = stage.tile([128, D], BF16, tag="xbf")
                    nc.vector.tensor_copy(xb[:], xs[:])
                    for half in range(2):
                        pt = pmisc.tile([128, 2, 128], BF16, tag="m")
                        for j in range(2):
                            kc = 2 * half + j
                            nc.tensor.transpose(pt[:, j, :],
                                                xb[:, kc * 128:(kc + 1) * 128],
                                                ident[:])
                        nc.vector.tensor_copy(
                            xT[:, 2 * half:2 * half + 2, nt * 128:(nt + 1) * 128],
                            pt[:])
                return emit

            # transpose only the rows pair-0 needs immediately; nt 8..15 are
            # folded into pair-0's attention loop below
            prelude = {
                0: [load_weight("Wv", Wv)],
                1: [load_weight("Wq", Wq)],
                3: [kt_unit(0, 0)],
                4: [v_unit(0)],
                5: [qt_unit(0, 0), v_unit(1)],
                7: [kt_unit(0, 1), qt_unit(0, 1)],
            }
            for nt in range(8):
                x_unit(nt)()
                for unit in prelude.get(nt, []):
                    unit()
            # weights not needed until mid/late attention load after the
            # critical prelude chain
            load_weight("Wg", Wg)()
            wo_b = wpool.tile([64, H, D], BF16)
            for h in range(H):
                ws = stage.tile([128, D], F32, tag="wostage", bufs=2)
                nc.sync.dma_start(ws[0:64, :], Wo[h * 64:(h + 1) * 64, :])
                nc.vector.tensor_copy(wo_b[:, h, :], ws[0:64, :])

            # during pair p's attention, emit projections for pair p+1
            # (v3 for the remaining jt is finished inside pair-0 ic=0,
            # pipelined two key-tiles ahead of its consumer)
            queues = {
                0: [gates_unit(1, 0), gates_unit(1, 1)]
                   + [qt_unit(1, ic) for ic in range(2)]
                   + [kt_unit(1, ic) for ic in range(4)],
                1: [gates_unit(2, 0), gates_unit(2, 1)]
                   + [qt_unit(2, ic) for ic in range(2)]
                   + [kt_unit(2, ic) for ic in range(4)],
                2: [gates_unit(3, 0), gates_unit(3, 1)]
                   + [qt_unit(3, ic) for ic in range(2)]
                   + [kt_unit(3, ic) for ic in range(4)],
                3: None,  # filled per-ic below: Wo/LN for it 0..3 during ic=1
            }

            # ---- attention, per head pair ----
            gatedT = acts.tile([64, H, NH], BF16)

            def wo_unit(it, psum="m", act_ln=False):
                def emit():
                    xres = stage.tile([128, D], F32, tag=f"xres{it % 4}")
                    nc.scalar.dma_start(xres[:], xkv[it * 128:(it + 1) * 128, :])
                    if psum == "att":
                        pw = papool.tile([128, 512], F32, tag="att")
                    elif psum == "pd":
                        pw_full = ppool.tile([128, 1024], F32, tag="pd")
                        pw = pw_full[:, 0:512]
                    else:
                        pw = pmisc.tile([128, 512], F32, tag="m")
                    for h in range(H):
                        nc.tensor.matmul(pw[:], gatedT[:, h, it * 128:(it + 1) * 128],
                                         wo_b[:, h, :], start=(h == 0),
                                         stop=(h == H - 1))
                    y = stage.tile([128, D], F32, tag="y")
                    nc.vector.tensor_add(y[:], pw[:], xres[:])
                    if not trivial_bo:
                        nc.vector.tensor_add(y[:], y[:], bo_b[:])
                    ve = stage.tile([128, 1], F32, tag="ve")
                    if act_ln:
                        # LN statistics on the (tail-idle) ScalarEngine:
                        # accum_out gives per-row sum / sum-of-squares
                        cp = stage.tile([128, D], F32, tag="gexp")
                        sm = stage.tile([128, 2], F32, tag="mv")
                        nc.scalar.activation(cp[:], y[:], AF.Copy,
                                             accum_out=sm[:, 0:1])
                        nc.scalar.activation(cp[:], y[:], AF.Square,
                                             accum_out=sm[:, 1:2])
                        mu = stage.tile([128, 1], F32, tag="muT")
                        nc.vector.tensor_scalar_mul(mu[:], sm[:, 0:1], 1.0 / D)
                        m2 = stage.tile([128, 1], F32, tag="m2T")
                        nc.vector.tensor_mul(m2[:], mu[:], mu[:])
                        nc.vector.tensor_scalar_mul(ve[:], sm[:, 1:2], 1.0 / D)
                        nc.vector.tensor_sub(ve[:], ve[:], m2[:])
                        nc.vector.tensor_add(ve[:], ve[:], eps_t[:])
                        mu_ap = mu[:]
                    else:
                        st = stage.tile([128, 6], F32, tag="st")
                        nc.vector.bn_stats(st[:], y[:])
                        mv = stage.tile([128, 2], F32, tag="mv")
                        nc.vector.bn_aggr(mv[:], st[:])
                        nc.vector.tensor_add(ve[:], mv[:, 1:2], eps_t[:])
                        mu_ap = mv[:, 0:1]
                    nc.vector.reciprocal(ve[:], ve[:])
                    nc.scalar.activation(ve[:], ve[:], AF.Sqrt)
                    z = stage.tile([128, D], F32, tag="z")
                    nc.vector.tensor_scalar(z[:], y[:], mu_ap, ve[:],
                                            OP.subtract, OP.mult)
                    if not trivial_gb:
                        nc.vector.tensor_mul(z[:], z[:], gam_b[:])
                        nc.vector.tensor_add(z[:], z[:], bet_b[:])
                    nc.sync.dma_start(out[it * 128:(it + 1) * 128, :], z[:])
                return emit

            for p in range(4):
                work = queues[p] or []
                wi = 0
                for ic in range(NH // 512):
                    if p == 3 and ic == 1:
                        work = [wo_unit(it) for it in range(4)]
                        wi = 0
                    pe_ = papool.tile([128, 512], F32, tag="att")
                    po_ = papool.tile([128, 512], F32, tag="att")

                    def dots_step(jt):
                        pd = ppool.tile([128, 1024], F32)
                        nc.tensor.matmul(pd[:, 0:512],
                                         kT[0:64, p, jt * 128:(jt + 1) * 128],
                                         qT[0:64, p, ic * 512:(ic + 1) * 512],
                                         start=True, stop=True,
                                         tile_position=(0, 0))
                        nc.tensor.matmul(pd[:, 512:1024],
                                         kT[64:128, p, jt * 128:(jt + 1) * 128],
                                         qT[64:128, p, ic * 512:(ic + 1) * 512],
                                         start=True, stop=True,
                                         tile_position=(64, 0))
                        return pd

                    # software pipeline: dots for jt+1 issue on the PE before
                    # the attnVs of jt, which wait on the exp of jt
                    pd_cur = dots_step(0)
                    for jt in range(NJT):
                        pr = prpool.tile([128, 2, 512], BF16, tag="pr")
                        nc.scalar.activation(
                            pr[:], pd_cur[:].rearrange("p (h x) -> p h x", h=2),
                            AF.Exp, scale=SCALE)
                        if jt + 1 < NJT:
                            pd_cur = dots_step(jt + 1)
                        if p == 0 and ic == 0:
                            if jt < 8:
                                x_unit(8 + jt)()
                            if jt == 4:
                                kt_unit(0, 2)()
                            elif jt == 8:
                                kt_unit(0, 3)()
                            if jt + 2 < NJT:
                                v_unit(jt + 2)()
                            elif jt == NJT - 2:
                                gates_unit(0, 0)()
                            else:
                                gates_unit(0, 1)()
                        elif wi < len(work) and (jt % 2 == 0 or wi > len(work) - 3):
                            work[wi]()
                            wi += 1
                        nc.tensor.matmul(pe_[0:65, :], v3[:, jt, 2 * p, :],
                                         pr[:, 0, :],
                                         start=(jt == 0), stop=(jt == NJT - 1))
                        nc.tensor.matmul(po_[0:65, :], v3[:, jt, 2 * p + 1, :],
                                         pr[:, 1, :],
                                         start=(jt == 0), stop=(jt == NJT - 1))
                    for hh, ph in ((2 * p, pe_), (2 * p + 1, po_)):
                        # evacuate PSUM fast (frees the accumulator bank for
                        # the next ic), then gate from SBUF off-critical-path
                        raw = stage.tile([65, 512], F32, tag="praw", bufs=4)
                        nc.vector.tensor_copy(raw[:], ph[0:65, :])
                        r0 = stage.tile([1, 512], F32, tag="r0")
                        nc.vector.reciprocal(r0[:], raw[64:65, :])
                        rb = stage.tile([64, 512], F32, tag="rb")
                        nc.gpsimd.partition_broadcast(rb[:], r0[:])
                        tmp = stage.tile([64, 512], F32, tag="tmp")
                        nc.vector.tensor_mul(tmp[:], raw[0:64, :], rb[:])
                        nc.vector.tensor_mul(gatedT[:, hh, ic * 512:(ic + 1) * 512],
                                             tmp[:], sigT[:, hh, ic * 512:(ic + 1) * 512])

            # ---- remaining Wo + LayerNorm tail units (it 4..7; 0..3 were
            #      interleaved into pair-3 attention). Three PSUM slots
            #      (pmisc/papool/ppool) keep the it-tiles pipelined. ----
            for it, ps in ((4, "m"), (5, "att"), (6, "pd"), (7, "m")):
                wo_unit(it, psum=ps, act_ln=True)()

    nc.compile()
    return nc


_NC_CACHE = {}


def _get_nc(trivial_bo=False, trivial_gb=False):
    key = (trivial_bo, trivial_gb)
    if key not in _NC_CACHE:
        _NC_CACHE[key] = build_nc(*key)
    return _NC_CACHE[key]


def kernel(**inputs) -> np.ndarray:
    x = np.asarray(inputs["x"], dtype=np.float32)
    Wq = np.ascontiguousarray(np.asarray(inputs["Wq"], dtype=np.float32))
    Wkv = np.asarray(inputs["Wkv"], dtype=np.float32)
    Wk = np.ascontiguousarray(Wkv[:, :D])
    Wv = np.ascontiguousarray(Wkv[:, D:])
    Wg = np.ascontiguousarray(np.asarray(inputs["Wg"], dtype=np.float32))
    Wo = np.ascontiguousarray(np.asarray(inputs["Wo"], dtype=np.float32))
    bg = np.ascontiguousarray(np.asarray(inputs["bg"], dtype=np.float32))
    bo = np.ascontiguousarray(np.asarray(inputs["bo"], dtype=np.float32))
    gamma = np.ascontiguousarray(np.asarray(inputs["gamma"], dtype=np.float32))
    beta = np.ascontiguousarray(np.asarray(inputs["beta"], dtype=np.float32))

    trivial_bo = bool(np.all(bo == 0.0))
    trivial_gb = bool(np.all(gamma == 1.0) and np.all(beta == 0.0))
    nc = _get_nc(trivial_bo, trivial_gb)
    in_maps = []
    for c in range(NCORES):
        b, half = c // 2, c % 2
        rolled = np.ascontiguousarray(np.roll(x[b], -half * NH, axis=0))
        in_maps.append({"xkv": rolled, "Wq": Wq, "Wk": Wk, "Wv": Wv,
                        "Wg": Wg, "Wo": Wo, "bg": bg, "bo": bo,
                        "gamma": gamma, "beta": beta})
    res = run_bass_kernel_spmd(nc, in_maps, core_ids=list(range(NCORES)))
    out = np.empty((B, N, D), dtype=np.float32)
    for c in range(NCORES):
        b, half = c // 2, c % 2
        out[b, half * NH:(half + 1) * NH] = res.results[c]["out"]
    return out


# revision 49
# speedup vs baseline: 1.0936x; 1.0031x over previous
"""Trainium2 (8 NeuronCores) kernel for a gated-attention transformer block.

Reference computation (per batch b):
    q = x@Wq, [k|v] = x@Wkv, heads=8, dh=64
    attn = softmax(q k^T / 8) v
    out  = (attn * sigmoid(x@Wg + bg)) @ Wo + bo + x
    out  = LayerNorm(out) * gamma + beta

Sharding: 8 cores = 4 batches x 2 sequence-halves. Each core computes
k/v for its full batch (duplicated across the half-pair; avoids any
collective) and q/gates/output for its own 1024 rows. Row order of
keys/values is irrelevant to attention, so each core receives x[b]
rolled so its own rows come first; compile-time indices are then
identical across cores (SPMD-safe).

On-chip layout: activations transposed ([feature, seq]) for projections
and attention; dots computed as dotsT[j, i] with 2x row-tiled matmuls
(K=64 head pairs on PE quadrants), softmax denominator via a ones-column
augmented attn@v matmul (M=65), gating + denominator applied in
transposed layout, final Wo projection back to natural layout for the
residual + LayerNorm tail. All matmuls bf16 with fp32 PSUM accumulation.

Scheduling: projections for head-pair p+1 are emitted interleaved with
attention of pair p so the TensorEngine stays busy while the ScalarEngine
runs the (bottleneck) softmax exponentials. All sigmoids are emitted
before the first exp and the LayerNorm sqrts after the last one, so the
ScalarEngine's activation table is switched exactly twice.
"""

import sys
import os
import numpy as np

for _p in ("/opt/trn_rl_repo", "/root/.axon_site/_ro/trn_rl_repo"):
    if os.path.isdir(_p) and _p not in sys.path:
        sys.path.insert(0, _p)

import concourse.bass as bass
import concourse.tile as tile
from concourse import bacc, mybir
from concourse.bass_utils import run_bass_kernel_spmd
from concourse.masks import make_identity

F32 = mybir.dt.float32
BF16 = mybir.dt.bfloat16
AF = mybir.ActivationFunctionType
OP = mybir.AluOpType

B, N, D, H, DH = 4, 2048, 512, 8, 64
NH = N // 2          # rows owned per core
NJT = N // 128       # 16 key tiles
SCALE = DH ** -0.5   # 0.125
EPS = 1e-5
NCORES = 8


def build_nc(trivial_bo=False, trivial_gb=False):
    nc = bacc.Bacc("TRN2", target_bir_lowering=False, debug=False,
                   num_devices=NCORES)

    xkv = nc.dram_tensor("xkv", [N, D], F32, kind="ExternalInput")
    Wq = nc.dram_tensor("Wq", [D, D], F32, kind="ExternalInput")
    Wk = nc.dram_tensor("Wk", [D, D], F32, kind="ExternalInput")
    Wv = nc.dram_tensor("Wv", [D, D], F32, kind="ExternalInput")
    Wg = nc.dram_tensor("Wg", [D, D], F32, kind="ExternalInput")
    Wo = nc.dram_tensor("Wo", [D, D], F32, kind="ExternalInput")
    bg = nc.dram_tensor("bg", [D], F32, kind="ExternalInput")
    bo = nc.dram_tensor("bo", [D], F32, kind="ExternalInput")
    gamma = nc.dram_tensor("gamma", [D], F32, kind="ExternalInput")
    beta = nc.dram_tensor("beta", [D], F32, kind="ExternalInput")
    out = nc.dram_tensor("out", [NH, D], F32, kind="ExternalOutput")

    def bcast_ap(t, n):
        return bass.AP(tensor=t, offset=0, ap=[[0, 128], [1, n]])

    with tile.TileContext(nc) as tc:
        with tc.tile_pool(name="consts", bufs=1) as consts, \
             tc.tile_pool(name="wpool", bufs=1) as wpool, \
             tc.tile_pool(name="acts", bufs=1) as acts, \
             tc.tile_pool(name="stage", bufs=2) as stage, \
             tc.tile_pool(name="prpool", bufs=6) as prpool, \
             tc.tile_pool(name="ppool", bufs=2, space="PSUM") as ppool, \
             tc.tile_pool(name="papool", bufs=2, space="PSUM") as papool, \
             tc.tile_pool(name="pmisc", bufs=2, space="PSUM") as pmisc:

            # ---- constants ----
            ident = consts.tile([128, 128], BF16)
            make_identity(nc, ident[:])
            bg_t = consts.tile([64, H], F32)
            nc.sync.dma_start(bg_t[:], bg.ap().rearrange("(h p) -> p h", p=64))
            bo_b = consts.tile([128, D], F32)
            nc.sync.dma_start(bo_b[:], bcast_ap(bo, D))
            gam_b = consts.tile([128, D], F32)
            nc.sync.dma_start(gam_b[:], bcast_ap(gamma, D))
            bet_b = consts.tile([128, D], F32)
            nc.sync.dma_start(bet_b[:], bcast_ap(beta, D))
            eps_t = consts.tile([128, 1], F32)
            nc.vector.memset(eps_t[:], EPS)

            # ---- weights: load fp32 in 128-row chunks, cast to bf16.
            #      Weight DMAs ride the sync queue; x DMAs ride the scalar
            #      queue so the two streams overlap. ----
            w_bf = {}

            def load_weight(name, t):
                def emit():
                    wb = wpool.tile([128, 4, D], BF16, tag=f"w_{name}")
                    for kc in range(4):
                        ws = stage.tile([128, D], F32, tag="wstage", bufs=4)
                        nc.sync.dma_start(ws[:], t[kc * 128:(kc + 1) * 128, :])
                        nc.vector.tensor_copy(wb[:, kc, :], ws[:])
                    w_bf[name] = wb
                return emit

            # Only Wk is loaded before the x pipeline: its cast is the one
            # the first dots transitively waits on (DVE executes in order).
            load_weight("Wk", Wk)()
            nbg = consts.tile([128, 4], F32)
            nc.sync.dma_start(nbg[:], bg.ap().rearrange("(m p) -> p m", p=128))
            nc.vector.tensor_scalar_mul(nbg[:], nbg[:], -1.0)

            # ---- tensors for x / projections ----
            xT = acts.tile([128, 4, N], BF16)
            sigT = acts.tile([64, H, NH], BF16)
            qT = acts.tile([128, 4, NH], BF16)
            kT = acts.tile([128, 4, N], BF16)
            v3 = acts.tile([128, NJT, H, DH + 1], BF16)
            nc.vector.memset(v3[:, :, :, DH:DH + 1], 1.0)

            def gates_unit(m, ic):
                # sigmoid(g+bg) = 1/(1+exp(-(g+bg))) -- uses the Exp table so
                # these can interleave freely with the attention exps
                def emit():
                    pm = pmisc.tile([128, 512], F32, tag="m")
                    for kc in range(4):
                        nc.tensor.matmul(pm[:], w_bf["Wg"][:, kc, m * 128:(m + 1) * 128],
                                         xT[:, kc, ic * 512:(ic + 1) * 512],
                                         start=(kc == 0), stop=(kc == 3))
                    e = stage.tile([128, 512], F32, tag="gexp")
                    nc.scalar.activation(e[:], pm[:], AF.Exp, scale=-1.0,
                                         bias=nbg[:, m:m + 1])
                    nc.vector.tensor_scalar_add(e[:], e[:], 1.0)
                    sp = stage.tile([128, 512], F32, tag="gsig")
                    nc.vector.reciprocal(sp[:], e[:])
                    nc.vector.tensor_copy(sigT[:, 2 * m, ic * 512:(ic + 1) * 512],
                                          sp[0:64, :])
                    nc.vector.tensor_copy(sigT[:, 2 * m + 1, ic * 512:(ic + 1) * 512],
                                          sp[64:128, :])
                return emit

            def qt_unit(m, ic):
                def emit():
                    pm = pmisc.tile([128, 512], F32, tag="m")
                    for kc in range(4):
                        nc.tensor.matmul(pm[:], w_bf["Wq"][:, kc, m * 128:(m + 1) * 128],
                                         xT[:, kc, ic * 512:(ic + 1) * 512],
                                         start=(kc == 0), stop=(kc == 3))
                    nc.vector.tensor_copy(qT[:, m, ic * 512:(ic + 1) * 512], pm[:])
                return emit

            def kt_unit(m, ic):
                def emit():
                    pm = pmisc.tile([128, 512], F32, tag="m")
                    for kc in range(4):
                        nc.tensor.matmul(pm[:], w_bf["Wk"][:, kc, m * 128:(m + 1) * 128],
                                         xT[:, kc, ic * 512:(ic + 1) * 512],
                                         start=(kc == 0), stop=(kc == 3))
                    nc.vector.tensor_copy(kT[:, m, ic * 512:(ic + 1) * 512], pm[:])
                return emit

            def v_unit(jt):
                def emit():
                    pm = pmisc.tile([128, 512], F32, tag="m")
                    for kc in range(4):
                        nc.tensor.matmul(pm[:], xT[:, kc, jt * 128:(jt + 1) * 128],
                                         w_bf["Wv"][:, kc, :],
                                         start=(kc == 0), stop=(kc == 3))
                    nc.vector.tensor_copy(
                        v3[:, jt, :, 0:DH],
                        pm[:].rearrange("p (h d) -> p h d", h=H))
                return emit

            # ---- x: load, cast, transpose; prelude projection units are
            #      emitted as soon as the xT columns they read exist, so
            #      gates/q/k/v overlap the transpose pipeline and attention
            #      can start while the tail of x is still being transposed.
            #      All sigmoids stay before the first exp (one table switch).
            def x_unit(nt):
                # two 128x128 transposes share one PSUM tile and one evac
                def emit():
                    xs = stage.tile([128, D], F32, tag="xstage", bufs=4)
                    dma_eng = nc.scalar if nt % 2 == 0 else nc.gpsimd
                    dma_eng.dma_start(xs[:], xkv[nt * 128:(nt + 1) * 128, :])
                    xb = stage.tile([128, D], BF16, tag="xbf")
                    nc.vector.tensor_copy(xb[:], xs[:])
                    for half in range(2):
                        pt = pmisc.tile([128, 2, 128], BF16, tag="m")
                        for j in range(2):
                            kc = 2 * half + j
                            nc.tensor.transpose(pt[:, j, :],
                                                xb[:, kc * 128:(kc + 1) * 128],
                                                ident[:])
                        # evacuate on the ScalarEngine: it idles during the
                        # x pipeline while the DVE is the serializing hop
                        # (Copy lives in every ACT table set - no switch)
                        nc.scalar.copy(
                            xT[:, 2 * half:2 * half + 2, nt * 128:(nt + 1) * 128],
                            pt[:])
                return emit

            # transpose only the rows pair-0 needs immediately; nt 8..15 are
            # folded into pair-0's attention loop below
            prelude = {
                0: [load_weight("Wv", Wv)],
                1: [load_weight("Wq", Wq)],
                3: [kt_unit(0, 0)],
                4: [v_unit(0)],
                5: [qt_unit(0, 0), v_unit(1)],
                7: [kt_unit(0, 1), qt_unit(0, 1)],
            }
            for nt in range(8):
                x_unit(nt)()
                for unit in prelude.get(nt, []):
                    unit()
            # weights not needed until mid/late attention load after the
            # critical prelude chain
            load_weight("Wg", Wg)()
            wo_b = wpool.tile([64, H, D], BF16)
            for h in range(H):
                ws = stage.tile([128, D], F32, tag="wostage", bufs=2)
                nc.sync.dma_start(ws[0:64, :], Wo[h * 64:(h + 1) * 64, :])
                nc.vector.tensor_copy(wo_b[:, h, :], ws[0:64, :])

            # during pair p's attention, emit projections for pair p+1
            # (v3 for the remaining jt is finished inside pair-0 ic=0,
            # pipelined two key-tiles ahead of its consumer)
            queues = {
                0: [gates_unit(1, 0), gates_unit(1, 1)]
                   + [qt_unit(1, ic) for ic in range(2)]
                   + [kt_unit(1, ic) for ic in range(4)],
                1: [gates_unit(2, 0), gates_unit(2, 1)]
                   + [qt_unit(2, ic) for ic in range(2)]
                   + [kt_unit(2, ic) for ic in range(4)],
                2: [gates_unit(3, 0), gates_unit(3, 1)]
                   + [qt_unit(3, ic) for ic in range(2)]
                   + [kt_unit(3, ic) for ic in range(4)],
                3: None,  # filled per-ic below: Wo/LN for it 0..3 during ic=1
            }

            # ---- attention, per head pair ----
            gatedT = acts.tile([64, H, NH], BF16)

            def wo_unit(it, psum="m", act_ln=False):
                def emit():
                    xres = stage.tile([128, D], F32, tag=f"xres{it % 4}")
                    nc.scalar.dma_start(xres[:], xkv[it * 128:(it + 1) * 128, :])
                    if psum == "att":
                        pw = papool.tile([128, 512], F32, tag="att")
                    elif psum == "pd":
                        pw_full = ppool.tile([128, 1024], F32, tag="pd")
                        pw = pw_full[:, 0:512]
                    else:
                        pw = pmisc.tile([128, 512], F32, tag="m")
                    for h in range(H):
                        nc.tensor.matmul(pw[:], gatedT[:, h, it * 128:(it + 1) * 128],
                                         wo_b[:, h, :], start=(h == 0),
                                         stop=(h == H - 1))
                    y = stage.tile([128, D], F32, tag="y")
                    nc.vector.tensor_add(y[:], pw[:], xres[:])
                    if not trivial_bo:
                        nc.vector.tensor_add(y[:], y[:], bo_b[:])
                    ve = stage.tile([128, 1], F32, tag="ve")
                    if act_ln:
                        # LN statistics on the (tail-idle) ScalarEngine:
                        # accum_out gives per-row sum / sum-of-squares
                        cp = stage.tile([128, D], F32, tag="gexp")
                        sm = stage.tile([128, 2], F32, tag="mv")
                        nc.scalar.activation(cp[:], y[:], AF.Copy,
                                             accum_out=sm[:, 0:1])
                        nc.scalar.activation(cp[:], y[:], AF.Square,
                                             accum_out=sm[:, 1:2])
                        mu = stage.tile([128, 1], F32, tag="muT")
                        nc.vector.tensor_scalar_mul(mu[:], sm[:, 0:1], 1.0 / D)
                        m2 = stage.tile([128, 1], F32, tag="m2T")
                        nc.vector.tensor_mul(m2[:], mu[:], mu[:])
                        nc.vector.tensor_scalar_mul(ve[:], sm[:, 1:2], 1.0 / D)
                        nc.vector.tensor_sub(ve[:], ve[:], m2[:])
                        nc.vector.tensor_add(ve[:], ve[:], eps_t[:])
                        mu_ap = mu[:]
                    else:
                        st = stage.tile([128, 6], F32, tag="st")
                        nc.vector.bn_stats(st[:], y[:])
                        mv = stage.tile([128, 2], F32, tag="mv")
                        nc.vector.bn_aggr(mv[:], st[:])
                        nc.vector.tensor_add(ve[:], mv[:, 1:2], eps_t[:])
                        mu_ap = mv[:, 0:1]
                    nc.vector.reciprocal(ve[:], ve[:])
                    nc.scalar.activation(ve[:], ve[:], AF.Sqrt)
                    z = stage.tile([128, D], F32, tag="z")
                    nc.vector.tensor_scalar(z[:], y[:], mu_ap, ve[:],
                                            OP.subtract, OP.mult)
                    if not trivial_gb:
                        nc.vector.tensor_mul(z[:], z[:], gam_b[:])
                        nc.vector.tensor_add(z[:], z[:], bet_b[:])
                    nc.sync.dma_start(out[it * 128:(it + 1) * 128, :], z[:])
                return emit

            for p in range(4):
                work = queues[p] or []
                wi = 0
                for ic in range(NH // 512):
                    if p == 3 and ic == 1:
                        work = [wo_unit(it) for it in range(4)]
                        wi = 0
                    pe_ = papool.tile([128, 512], F32, tag="att")
                    po_ = papool.tile([128, 512], F32, tag="att")

                    def dots_step(jt):
                        pd = ppool.tile([128, 1024], F32)
                        nc.tensor.matmul(pd[:, 0:512],
                                         kT[0:64, p, jt * 128:(jt + 1) * 128],
                                         qT[0:64, p, ic * 512:(ic + 1) * 512],
                                         start=True, stop=True,
                                         tile_position=(0, 0))
                        nc.tensor.matmul(pd[:, 512:1024],
                                         kT[64:128, p, jt * 128:(jt + 1) * 128],
                                         qT[64:128, p, ic * 512:(ic + 1) * 512],
                                         start=True, stop=True,
                                         tile_position=(64, 0))
                        return pd

                    # software pipeline: dots for jt+1 issue on the PE before
                    # the attnVs of jt, which wait on the exp of jt
                    pd_cur = dots_step(0)
                    for jt in range(NJT):
                        pr = prpool.tile([128, 2, 512], BF16, tag="pr")
                        nc.scalar.activation(
                            pr[:], pd_cur[:].rearrange("p (h x) -> p h x", h=2),
                            AF.Exp, scale=SCALE)
                        if jt + 1 < NJT:
                            pd_cur = dots_step(jt + 1)
                        if p == 0 and ic == 0:
                            if jt < 8:
                                x_unit(8 + jt)()
                            if jt == 4:
                                kt_unit(0, 2)()
                            elif jt == 8:
                                kt_unit(0, 3)()
                            if jt + 2 < NJT:
                                v_unit(jt + 2)()
                            elif jt == NJT - 2:
                                gates_unit(0, 0)()
                            else:
                                gates_unit(0, 1)()
                        elif wi < len(work) and (jt % 2 == 0 or wi > len(work) - 3):
                            work[wi]()
                            wi += 1
                        nc.tensor.matmul(pe_[0:65, :], v3[:, jt, 2 * p, :],
                                         pr[:, 0, :],
                                         start=(jt == 0), stop=(jt == NJT - 1))
                        nc.tensor.matmul(po_[0:65, :], v3[:, jt, 2 * p + 1, :],
                                         pr[:, 1, :],
                                         start=(jt == 0), stop=(jt == NJT - 1))
                    for hh, ph in ((2 * p, pe_), (2 * p + 1, po_)):
                        # evacuate PSUM fast (frees the accumulator bank for
                        # the next ic), then gate from SBUF off-critical-path
                        raw = stage.tile([65, 512], F32, tag="praw", bufs=4)
                        nc.vector.tensor_copy(raw[:], ph[0:65, :])
                        r0 = stage.tile([1, 512], F32, tag="r0")
                        nc.vector.reciprocal(r0[:], raw[64:65, :])
                        rb = stage.tile([64, 512], F32, tag="rb")
                        nc.gpsimd.partition_broadcast(rb[:], r0[:])
                        tmp = stage.tile([64, 512], F32, tag="tmp")
                        nc.vector.tensor_mul(tmp[:], raw[0:64, :], rb[:])
                        nc.vector.tensor_mul(gatedT[:, hh, ic * 512:(ic + 1) * 512],
                                             tmp[:], sigT[:, hh, ic * 512:(ic + 1) * 512])

            # ---- remaining Wo + LayerNorm tail units (it 4..7; 0..3 were
            #      interleaved into pair-3 attention). Three PSUM slots
            #      (pmisc/papool/ppool) keep the it-tiles pipelined. ----
            for it, ps in ((4, "m"), (5, "att"), (6, "pd"), (7, "m")):
                wo_unit(it, psum=ps, act_ln=True)()

    nc.compile()
    return nc


_NC_CACHE = {}


def _get_nc(trivial_bo=False, trivial_gb=False):
    key = (trivial_bo, trivial_gb)
    if key not in _NC_CACHE:
        _NC_CACHE[key] = build_nc(*key)
    return _NC_CACHE[key]


def kernel(**inputs) -> np.ndarray:
    x = np.asarray(inputs["x"], dtype=np.float32)
    Wq = np.ascontiguousarray(np.asarray(inputs["Wq"], dtype=np.float32))
    Wkv = np.asarray(inputs["Wkv"], dtype=np.float32)
    Wk = np.ascontiguousarray(Wkv[:, :D])
    Wv = np.ascontiguousarray(Wkv[:, D:])
    Wg = np.ascontiguousarray(np.asarray(inputs["Wg"], dtype=np.float32))
    Wo = np.ascontiguousarray(np.asarray(inputs["Wo"], dtype=np.float32))
    bg = np.ascontiguousarray(np.asarray(inputs["bg"], dtype=np.float32))
    bo = np.ascontiguousarray(np.asarray(inputs["bo"], dtype=np.float32))
    gamma = np.ascontiguousarray(np.asarray(inputs["gamma"], dtype=np.float32))
    beta = np.ascontiguousarray(np.asarray(inputs["beta"], dtype=np.float32))

    trivial_bo = bool(np.all(bo == 0.0))
    trivial_gb = bool(np.all(gamma == 1.0) and np.all(beta == 0.0))
    nc = _get_nc(trivial_bo, trivial_gb)
    in_maps = []
    for c in range(NCORES):
        b, half = c // 2, c % 2
        rolled = np.ascontiguousarray(np.roll(x[b], -half * NH, axis=0))
        in_maps.append({"xkv": rolled, "Wq": Wq, "Wk": Wk, "Wv": Wv,
                        "Wg": Wg, "Wo": Wo, "bg": bg, "bo": bo,
                        "gamma": gamma, "beta": beta})
    res = run_bass_kernel_spmd(nc, in_maps, core_ids=list(range(NCORES)))
    out = np.empty((B, N, D), dtype=np.float32)
    for c in range(NCORES):
        b, half = c // 2, c % 2
        out[b, half * NH:(half + 1) * NH] = res.results[c]["out"]
    return out
